# revision 8
# baseline (speedup 1.0000x reference)
"""ViT self-attention (B=32, S=577, D=1024, H=16, Dh=64) on 8 TRN2 NeuronCores.

Sharding: data-parallel over batch — each core gets 4 batch elements, no
collectives.

All matmuls run in bf16 (fp32 matmul is 4 cycles/row vs 1 for bf16; tolerance
2e-2 leaves ample room). The host passes hidden_states pre-converted to bf16
and zero-padded to 640 tokens, plus bf16 weights, so:
  phase 0: X^T comes straight from XBAR DMA-transpose loads (no PE transposes)
  phase 1: Q^T = Wq^T X^T, K^T = Wk^T X^T (bias folded into DVE evac, bf16),
           V natural = X Wv with bv folded in: v = [V_h + bv_h | 1] bf16 per
           head (ones column makes the softmax denominator fall out of the
           ctx matmul; (ctx + den*bv)/den = ctx/den + bv)
  phase 2: per head pair (row-packed K=64 matmuls at tile_position (0,0)/(64,0)):
           S^T tile = matmul(lhsT=K^T, rhs=Q^T); P^T = exp(S^T/8) on ACT (bf16);
           ctx natural = matmul(lhsT=P^T, rhs=[V+bv|1]) accumulated in PSUM;
           DVE: reciprocal of denominator + scale evac.
  phase 3: DMA out per head pair as soon as its ctx is evacuated.

Cross-batch software pipeline: attention of batch b (ACT-paced: exp drains
PSUM at ~154G elem/s) is woven with QKV matmuls of batch b+1 (pure PE) at
unit granularity so neither engine starves.
"""

import numpy as np

import concourse.bass as bass
import concourse.mybir as mybir
import concourse.tile as tile
from concourse.bass import ds, ts
from concourse.bass_utils import run_bass_kernel_spmd
from concourse.masks import make_identity

F32 = mybir.dt.float32
BF16 = mybir.dt.bfloat16

# ---------------------------------------------------------------------------
# Wait-legalization patch: this walrus build accepts at most ONE ge-mode sync
# wait per instruction (eq-mode counts as two). Tile's sem assignment attaches
# multi-waits directly to instructions, so hoist extras onto standalone
# EventSemaphore carriers (same engine queue, immediately preceding — identical
# semantics, queue is in-order).
# ---------------------------------------------------------------------------
_ctr = [0]


def _split_waits(insts):
    out = []
    for inst in insts:
        si = inst.sync_info
        if si is not None and si.on_wait:
            waits = list(si.on_wait)
            if len(waits) == 1 and waits[0].wait_mode != "sem-eq-imm":
                move = []
            else:
                move = waits
            for w in move:
                _ctr[0] += 1
                ev = mybir.InstEventSemaphore(
                    name=f"wsplit_{_ctr[0]}", opcode="EventSemaphore",
                    engine=inst.engine, debug=inst.debug, ins=[], outs=[],
                    sync_info=mybir.SyncInfo(on_wait=[w], on_update=[]),
                )
                out.append(ev)
            if move:
                inst.sync_info = mybir.SyncInfo(on_wait=[], on_update=list(si.on_update))
        out.append(inst)
    return out


def _install_waitfix():
    if getattr(tile.TileContext, "_waitfix_installed", False):
        return
    from concourse.vector_clock import ScopedClock

    orig_lower = tile.TileContext._lower_ordered_insts

    def patched_lower(self, ordered):
        for name in list(ordered.keys()):
            ordered[name] = _split_waits(ordered[name])
        return orig_lower(self, ordered)

    def patched_dab(self, tick_clock, wait_clock):
        nc = self.nc
        probe = nc.sync.nop(nofuse=True)
        wait_clock.add_sem_waits(probe.ins, ScopedClock({None: tick_clock.global_clock}))
        si = probe.ins.sync_info
        waits = list(si.on_wait) if si is not None else []
        probe.ins.sync_info = mybir.SyncInfo(
            on_wait=[], on_update=list(si.on_update) if si else []
        )
        for w in waits:
            _ctr[0] += 1
            ev = mybir.InstEventSemaphore(
                name=f"wsplit_dab_{_ctr[0]}", opcode="EventSemaphore",
                engine=mybir.EngineType.SP, debug=probe.ins.debug, ins=[], outs=[],
                sync_info=mybir.SyncInfo(on_wait=[w], on_update=[]),
            )
            nc.sync.add_instruction(ev)
        nc.sync.drain()
        nc.all_engine_barrier()
        assert self.sems is not None
        popped = nc._tile_sem_poison_stack.pop()
        assert popped is self._sem_poison
        nc.clear_and_free_semaphores(list(self.sems.allocated().values()))
        nc.all_engine_barrier()

    tile.TileContext._lower_ordered_insts = patched_lower
    tile.TileContext._drain_and_barrier = patched_dab
    tile.TileContext._waitfix_installed = True


_install_waitfix()

N_CORES = 8
B, S, D = 32, 577, 1024
H, Dh = 16, 64
BPC = B // N_CORES  # batches per core
SP_ = 640           # padded token count (multiple of 128, for XBAR transpose)
S_TILES = [(t * 128, min(128, S - t * 128)) for t in range((S + 127) // 128)]  # 5
NT = len(S_TILES)
ND = D // 128  # 8 din/dout tiles
HPAIRS = H // 2
SB = S          # token-block stride inside fused Q^T/K^T tiles
XB = SP_        # token-block stride inside the X^T tile
VB = H * 65     # v-block stride ([V_h+bv|1] x 16 heads)

AF = mybir.ActivationFunctionType
OP = mybir.AluOpType


def build_nc():
    nc = bass.Bass()
    hidden = nc.declare_dram_parameter("hidden", [BPC, SP_, D], BF16, isOutput=False)
    wq = nc.declare_dram_parameter("Wq", [D, D], BF16, isOutput=False)
    bq = nc.declare_dram_parameter("bq", [D], F32, isOutput=False)
    wk = nc.declare_dram_parameter("Wk", [D, D], BF16, isOutput=False)
    bk = nc.declare_dram_parameter("bk", [D], F32, isOutput=False)
    wv = nc.declare_dram_parameter("Wv", [D, D], BF16, isOutput=False)
    bv = nc.declare_dram_parameter("bv", [D], F32, isOutput=False)
    out = nc.declare_dram_parameter("out", [BPC, S, D], F32, isOutput=True)

    with tile.TileContext(nc) as tc:
        with (
            tc.tile_pool(name="singles", bufs=1) as singles,
            tc.tile_pool(name="wbf", bufs=1) as wbf_pool,
            tc.tile_pool(name="xt", bufs=2) as xt_pool,
            tc.tile_pool(name="qkt", bufs=2) as qkt_pool,
            tc.tile_pool(name="v", bufs=2) as v_pool,
            tc.tile_pool(name="pT", bufs=40) as pT_pool,
            tc.tile_pool(name="ost", bufs=5) as o_pool,
            tc.tile_pool(name="rc", bufs=6) as rc_pool,
            tc.tile_pool(name="psbig", bufs=3, space="PSUM") as ps_big,
            tc.tile_pool(name="psctx", bufs=2, space="PSUM") as ps_ctx,
        ):
            # --- constants ---
            # per-dout-tile bias columns: bqt[:, m] = bq[128m : 128(m+1)]
            bqt = singles.tile([128, ND], F32)
            bkt = singles.tile([128, ND], F32)
            nc.gpsimd.dma_start(out=bqt, in_=bq[:].rearrange("(m p) -> p m", p=128))
            nc.gpsimd.dma_start(out=bkt, in_=bk[:].rearrange("(m p) -> p m", p=128))
            # bv broadcast to all 128 partitions (folded into V at evac)
            bvb = singles.tile([128, D], F32)
            bv_ap = bv[:]
            nc.gpsimd.dma_start(
                out=bvb,
                in_=bass.AP(tensor=bv_ap.tensor, offset=bv_ap.offset, ap=[[0, 128]] + bv_ap.ap),
            )

            # --- weights: already bf16 in DRAM, plain loads ---
            # [128, 512] half-tiles so the first Q matmuls can start after only
            # ~1 MB of wire; configs round-robin between the two HWDGE queues.
            wbf = {}
            for wname, wdram in (("q", wq), ("k", wk), ("v", wv)):
                for k in range(ND):
                    wt = wbf_pool.tile([128, D], BF16, tag=f"w{wname}{k}", name=f"w{wname}{k}")
                    wbf[(wname, k)] = wt

            def emit_w_loads():
                wq_engines = [nc.scalar, nc.sync]
                wi = 0
                for wname, wdram in (("q", wq), ("k", wk), ("v", wv)):
                    for half in range(2):
                        for k in range(ND):
                            eng = wq_engines[wi % 2]
                            wi += 1
                            eng.dma_start(
                                out=wbf[(wname, k)][:, ds(half * 512, 512)],
                                in_=wdram[ts(k, 128), ds(half * 512, 512)],
                            )

            state = {}

            def st_of(b):
                if b not in state:
                    state[b] = {"pT": {}}
                return state[b]

            # ---------- qkv units (one closure ≈ 0.9-2 µs of PE work) ----------
            def u_xt(b):
                # X^T straight from DRAM via XBAR DMA-transpose, one per din
                # block j (HWDGE-only: SP or ACT queues).
                def emit():
                    st = st_of(b)
                    st["xt"] = xt_pool.tile([128, ND * XB], BF16, tag="xt", name=f"xt{b}")
                    for j in range(ND):
                        nc.sync.dma_start_transpose(
                            out=st["xt"][:, ds(j * XB, XB)],
                            in_=hidden[b, :, ts(j, 128)],
                        )
                return emit

            def u_qk(b, which, m, part=None):
                # part=None: whole m-tile. part=0: [0:512] accumulation only
                # (allocates the psum tile). part=1: [512:S] accumulation +
                # bias evac. Splitting gives the weave finer PE granularity.
                def emit():
                    st = st_of(b)
                    key = "qt" if which == "q" else "kt"
                    if key not in st:
                        st[key] = qkt_pool.tile(
                            [128, ND * SB], BF16, tag=key, name=f"{key}{b}"
                        )
                    dst = st[key]
                    bias = bqt if which == "q" else bkt
                    xt = st["xt"]
                    if part in (None, 0):
                        ps = ps_big.tile([128, 1024], F32, tag="big", name="psbig")
                        st[("qkps", which, m)] = ps
                        for k in range(ND):
                            nc.tensor.matmul(
                                ps[:, 0:512], wbf[(which, k)][:, ts(m, 128)],
                                xt[:, ds(k * XB, 512)],
                                start=(k == 0), stop=(k == ND - 1),
                            )
                    if part in (None, 1):
                        ps = st.pop(("qkps", which, m))
                        for k in range(ND):
                            nc.tensor.matmul(
                                ps[:, 512:S], wbf[(which, k)][:, ts(m, 128)],
                                xt[:, ds(k * XB + 512, S - 512)],
                                start=(k == 0), stop=(k == ND - 1),
                            )
                        nc.vector.tensor_scalar_add(
                            dst[:, ds(m * SB, SB)], ps[:, 0:S], bias[:, m : m + 1]
                        )
                return emit

            def u_v(b, t, part=None):
                # part=0: [0:512] accumulation (allocates psum). part=1:
                # [512:1024] accumulation + bv-fold evac. None: both.
                def emit():
                    st = st_of(b)
                    if "v" not in st:
                        st["v"] = v_pool.tile([128, NT * VB], BF16, tag="v", name=f"v{b}")
                    t0, sz = S_TILES[t]
                    xt = st["xt"]
                    if part in (None, 0):
                        ps = ps_big.tile([128, 1024], F32, tag="big", name="psbig")
                        st[("vps", t)] = ps
                        for k in range(ND):
                            nc.tensor.matmul(
                                ps[:sz, 0:512], xt[:, ds(k * XB + t0, sz)],
                                wbf[("v", k)][:, 0:512],
                                start=(k == 0), stop=(k == ND - 1),
                            )
                    if part in (None, 1):
                        ps = st.pop(("vps", t))
                        for k in range(ND):
                            nc.tensor.matmul(
                                ps[:sz, 512:1024], xt[:, ds(k * XB + t0, sz)],
                                wbf[("v", k)][:, 512:1024],
                                start=(k == 0), stop=(k == ND - 1),
                            )
                        v3 = st["v"][:, ds(t * VB, VB)].rearrange("p (h c) -> p h c", c=65)
                        nc.vector.tensor_tensor(
                            out=v3[:sz, :, 0:64],
                            in0=ps[:sz].rearrange("p (h c) -> p h c", c=64),
                            in1=bvb[:sz].rearrange("p (h c) -> p h c", c=64),
                            op=OP.add,
                        )
                        nc.vector.memset(v3[:, :, 64:65], 1.0)
                return emit

            def qhead_units(b):
                # next batch's X^T load + Q projection: pure-PE filler with no
                # ACT coupling, woven into the previous batch's attn window.
                return [u_xt(b)] + [u_qk(b, "q", m) for m in range(ND)]

            # ---------- attention units ----------
            def u_sc(b, p, t, half):
                def emit():
                    st = st_of(b)
                    t0, sz = S_TILES[t]
                    h0 = half * 64
                    qt, kt = st["qt"], st["kt"]
                    ps = ps_big.tile([128, 1024], F32, tag="big", name="psbig")
                    nc.tensor.matmul(
                        ps[:sz, 0:512],
                        kt[h0 : h0 + 64, ds(p * SB + t0, sz)],
                        qt[h0 : h0 + 64, ds(p * SB, 512)],
                        start=True, stop=True, tile_position=(h0, 0),
                    )
                    nc.tensor.matmul(
                        ps[:sz, 512:S],
                        kt[h0 : h0 + 64, ds(p * SB + t0, sz)],
                        qt[h0 : h0 + 64, ds(p * SB + 512, S - 512)],
                        start=True, stop=True, tile_position=(h0, 0),
                    )
                    pT = pT_pool.tile([128, SB], BF16, tag="pT", name="pT")
                    nc.scalar.activation(pT[:sz], ps[:sz, 0:S], AF.Exp, scale=0.125)
                    st["pT"][(p, half, t)] = pT
                return emit

            def u_ctx(b, p, half=None):
                halves = (0, 1) if half is None else (half,)

                def emit():
                    st = st_of(b)
                    if "ost" not in st:
                        st["ost"] = [
                            o_pool.tile([128, D], F32, tag="ost", name=f"ost{b}_{j}")
                            for j in range(NT)
                        ]
                    for half_ in halves:
                        h = 2 * p + half_
                        psc = ps_ctx.tile([128, 512], F32, tag="ctx", name="psctx")
                        for j, (j0, sj) in enumerate(S_TILES):
                            for t, (t0, szt) in enumerate(S_TILES):
                                pT = st["pT"][(p, half_, t)]
                                nc.tensor.matmul(
                                    psc[:sj, ds(65 * j, 65)],
                                    pT[:szt, j0 : j0 + sj],
                                    st["v"][:szt, ds(t * VB + 65 * h, 65)],
                                    start=(t == 0), stop=(t == NT - 1),
                                )
                        rc = rc_pool.tile([128, 8], F32, tag="rc", name="rc")
                        den = psc[:, 0:325].rearrange("p (j c) -> p j c", c=65)
                        nc.vector.reciprocal(
                            rc[:, 0:4].rearrange("p (j c) -> p j c", c=1),
                            den[:, 0:4, 64:65],
                        )
                        nc.vector.reciprocal(
                            rc[:65, 4:5].rearrange("p (j c) -> p j c", c=1),
                            den[:65, 4:5, 64:65],
                        )
                        for j, (j0, sj) in enumerate(S_TILES):
                            nc.vector.tensor_scalar_mul(
                                st["ost"][j][:sj, ds(64 * h, 64)],
                                psc[:sj, ds(65 * j, 64)],
                                rc[:sj, j : j + 1],
                            )
                    if halves[-1] == 1:
                        # stream this head pair's 128 output columns out now
                        for j, (j0, sj) in enumerate(S_TILES):
                            nc.sync.dma_start(
                                out=out[b, j0 : j0 + sj, ds(128 * p, 128)],
                                in_=st["ost"][j][:sj, ds(128 * p, 128)],
                            )
                return emit

            def qkv_units(b, include_xt=True):
                # full projection window (batches 0..BPC-2)
                units = [u_xt(b)] if include_xt else []
                units += [u_qk(b, "q", m) for m in range(ND)]
                units += [u_qk(b, "k", m) for m in range(ND)]
                units += [u_v(b, t) for t in range(NT)]
                return units

            def attn_units(b):
                # middle-window attention: scores groups with 1-pair ctx delay
                units = []
                for p in range(HPAIRS):
                    for t in range(NT):
                        for half in range(2):
                            units.append(u_sc(b, p, t, half))
                    if p >= 1:
                        units.append(u_ctx(b, p - 1))
                units.append(u_ctx(b, HPAIRS - 1))
                return units

            # ---------- proportional weave, preserving per-list order ----------
            def weave(a_units, q_units):
                merged = []
                ia = iq = 0
                la, lq = len(a_units), len(q_units)
                while ia < la or iq < lq:
                    if iq < lq and (ia >= la or (iq + 1) * la <= (ia + 1) * lq):
                        merged.append(q_units[iq])
                        iq += 1
                    else:
                        merged.append(a_units[ia])
                        ia += 1
                return merged

            def astar_last(b):
                # final window: no next batch to weave, so this batch's own
                # K/V projections (deferred from its qkv window) act as PE
                # filler between score groups, split fine-grained.
                units = [u_qk(b, "k", 0, 0), u_qk(b, "k", 0, 1)]
                for p in range(HPAIRS):
                    scs = []
                    for t in range(NT):
                        for half in range(2):
                            scs.append(u_sc(b, p, t, half))
                    fillers = []
                    if p == 0:
                        fillers += [u_v(b, 0, 0), u_v(b, 0, 1),
                                    u_v(b, 1, 0), u_v(b, 1, 1)]
                    elif p == 1:
                        fillers += [u_v(b, 2, 0), u_v(b, 2, 1),
                                    u_v(b, 3, 0), u_v(b, 3, 1)]
                    elif p == 2:
                        fillers += [u_v(b, 4, 0), u_v(b, 4, 1)]
                    if p + 1 < HPAIRS:
                        fillers += [u_qk(b, "k", p + 1, 0), u_qk(b, "k", p + 1, 1)]
                    if p >= 3:
                        fillers += [u_ctx(b, p - 3, 0), u_ctx(b, p - 3, 1)]
                    units += weave(scs, fillers)
                for p in range(HPAIRS - 3, HPAIRS):
                    units += [u_ctx(b, p, 0), u_ctx(b, p, 1)]
                return units

            # ---------- emission ----------
            last = BPC - 1
            emit_w_loads()
            for b in range(BPC):
                if b < last:
                    attn = attn_units(b - 1) if b >= 1 else []
                    for u in weave(attn, qkv_units(b)):
                        u()
                else:
                    for u in weave(attn_units(b - 1), qhead_units(b)):
                        u()
            for u in astar_last(last):
                u()

    return nc


_NC = None


def prep_in_maps(hidden_states, Wq, bq, Wk, bk, Wv, bv):
    """Host-side prep: hidden -> bf16 zero-padded to 640 tokens; weights -> bf16."""
    import ml_dtypes

    bf16 = ml_dtypes.bfloat16
    hs = np.asarray(hidden_states, dtype=np.float32)
    hb = np.zeros((B, SP_, D), dtype=bf16)
    hb[:, :S, :] = hs.astype(bf16)
    args = {
        "Wq": np.ascontiguousarray(np.asarray(Wq, np.float32).astype(bf16)),
        "bq": np.ascontiguousarray(np.asarray(bq, np.float32)),
        "Wk": np.ascontiguousarray(np.asarray(Wk, np.float32).astype(bf16)),
        "bk": np.ascontiguousarray(np.asarray(bk, np.float32)),
        "Wv": np.ascontiguousarray(np.asarray(Wv, np.float32).astype(bf16)),
        "bv": np.ascontiguousarray(np.asarray(bv, np.float32)),
    }
    return [
        {"hidden": hb[i * BPC : (i + 1) * BPC], **args} for i in range(N_CORES)
    ]


def kernel(hidden_states, Wq, bq, Wk, bk, Wv, bv):
    global _NC
    if _NC is None:
        _NC = build_nc()
    in_maps = prep_in_maps(hidden_states, Wq, bq, Wk, bk, Wv, bv)
    res = run_bass_kernel_spmd(_NC, in_maps, list(range(N_CORES)))
    return np.concatenate([res.results[i]["out"] for i in range(N_CORES)], axis=0)


# revision 10
# speedup vs baseline: 1.0770x; 1.0770x over previous
"""ViT self-attention (B=32, S=577, D=1024, H=16, Dh=64) on 8 TRN2 NeuronCores.

Sharding: data-parallel over batch — each core gets 4 batch elements, no
collectives.

All matmuls run in bf16 (fp32 matmul is 4 cycles/row vs 1 for bf16; tolerance
2e-2 leaves ample room). The host passes hidden_states pre-converted to bf16
and zero-padded to 640 tokens, plus bf16 weights, so:
  phase 0: X^T comes straight from XBAR DMA-transpose loads (no PE transposes)
  phase 1: Q^T = Wq^T X^T, K^T = Wk^T X^T (bias folded into DVE evac, bf16),
           V natural = X Wv with bv folded in: v = [V_h + bv_h | 1] bf16 per
           head (ones column makes the softmax denominator fall out of the
           ctx matmul; (ctx + den*bv)/den = ctx/den + bv)
  phase 2: per head pair (row-packed K=64 matmuls at tile_position (0,0)/(64,0)):
           S^T tile = matmul(lhsT=K^T, rhs=Q^T); P^T = exp(S^T/8) on ACT (bf16);
           ctx natural = matmul(lhsT=P^T, rhs=[V+bv|1]) accumulated in PSUM;
           DVE: reciprocal of denominator + scale evac.
  phase 3: DMA out per head pair as soon as its ctx is evacuated.

Cross-batch software pipeline: attention of batch b (ACT-paced: exp drains
PSUM at ~154G elem/s) is woven with QKV matmuls of batch b+1 (pure PE) at
unit granularity so neither engine starves.
"""

import numpy as np

import concourse.bass as bass
import concourse.mybir as mybir
import concourse.tile as tile
from concourse.bass import ds, ts
from concourse.bass_utils import run_bass_kernel_spmd
from concourse.masks import make_identity

F32 = mybir.dt.float32
BF16 = mybir.dt.bfloat16

# ---------------------------------------------------------------------------
# Wait-legalization patch: this walrus build accepts at most ONE ge-mode sync
# wait per instruction (eq-mode counts as two). Tile's sem assignment attaches
# multi-waits directly to instructions, so hoist extras onto standalone
# EventSemaphore carriers (same engine queue, immediately preceding — identical
# semantics, queue is in-order).
# ---------------------------------------------------------------------------
_ctr = [0]


def _split_waits(insts):
    out = []
    for inst in insts:
        si = inst.sync_info
        if si is not None and si.on_wait:
            waits = list(si.on_wait)
            if len(waits) == 1 and waits[0].wait_mode != "sem-eq-imm":
                move = []
            else:
                move = waits
            for w in move:
                _ctr[0] += 1
                ev = mybir.InstEventSemaphore(
                    name=f"wsplit_{_ctr[0]}", opcode="EventSemaphore",
                    engine=inst.engine, debug=inst.debug, ins=[], outs=[],
                    sync_info=mybir.SyncInfo(on_wait=[w], on_update=[]),
                )
                out.append(ev)
            if move:
                inst.sync_info = mybir.SyncInfo(on_wait=[], on_update=list(si.on_update))
        out.append(inst)
    return out


def _install_waitfix():
    if getattr(tile.TileContext, "_waitfix_installed", False):
        return
    from concourse.vector_clock import ScopedClock

    orig_lower = tile.TileContext._lower_ordered_insts

    def patched_lower(self, ordered):
        for name in list(ordered.keys()):
            ordered[name] = _split_waits(ordered[name])
        return orig_lower(self, ordered)

    def patched_dab(self, tick_clock, wait_clock):
        nc = self.nc
        probe = nc.sync.nop(nofuse=True)
        wait_clock.add_sem_waits(probe.ins, ScopedClock({None: tick_clock.global_clock}))
        si = probe.ins.sync_info
        waits = list(si.on_wait) if si is not None else []
        probe.ins.sync_info = mybir.SyncInfo(
            on_wait=[], on_update=list(si.on_update) if si else []
        )
        for w in waits:
            _ctr[0] += 1
            ev = mybir.InstEventSemaphore(
                name=f"wsplit_dab_{_ctr[0]}", opcode="EventSemaphore",
                engine=mybir.EngineType.SP, debug=probe.ins.debug, ins=[], outs=[],
                sync_info=mybir.SyncInfo(on_wait=[w], on_update=[]),
            )
            nc.sync.add_instruction(ev)
        nc.sync.drain()
        nc.all_engine_barrier()
        assert self.sems is not None
        popped = nc._tile_sem_poison_stack.pop()
        assert popped is self._sem_poison
        nc.clear_and_free_semaphores(list(self.sems.allocated().values()))
        nc.all_engine_barrier()

    tile.TileContext._lower_ordered_insts = patched_lower
    tile.TileContext._drain_and_barrier = patched_dab
    tile.TileContext._waitfix_installed = True


_install_waitfix()

N_CORES = 8
B, S, D = 32, 577, 1024
H, Dh = 16, 64
BPC = B // N_CORES  # batches per core
SP_ = 640           # padded token count (multiple of 128, for XBAR transpose)
S_TILES = [(t * 128, min(128, S - t * 128)) for t in range((S + 127) // 128)]  # 5
NT = len(S_TILES)
ND = D // 128  # 8 din/dout tiles
HPAIRS = H // 2
SB = S          # token-block stride inside fused Q^T/K^T tiles
XB = SP_        # token-block stride inside the X^T tile
VB = H * 65     # v-block stride ([V_h+bv|1] x 16 heads)

AF = mybir.ActivationFunctionType
OP = mybir.AluOpType


def build_nc():
    nc = bass.Bass()
    hidden = nc.declare_dram_parameter("hidden", [BPC, SP_, D], BF16, isOutput=False)
    wq = nc.declare_dram_parameter("Wq", [D, D], BF16, isOutput=False)
    bq = nc.declare_dram_parameter("bq", [D], F32, isOutput=False)
    wk = nc.declare_dram_parameter("Wk", [D, D], BF16, isOutput=False)
    bk = nc.declare_dram_parameter("bk", [D], F32, isOutput=False)
    wv = nc.declare_dram_parameter("Wv", [D, D], BF16, isOutput=False)
    bv = nc.declare_dram_parameter("bv", [D], F32, isOutput=False)
    out = nc.declare_dram_parameter("out", [BPC, S, D], F32, isOutput=True)

    with tile.TileContext(nc) as tc:
        with (
            tc.tile_pool(name="singles", bufs=1) as singles,
            tc.tile_pool(name="wbf", bufs=1) as wbf_pool,
            tc.tile_pool(name="xt", bufs=2) as xt_pool,
            tc.tile_pool(name="qkt", bufs=2) as qkt_pool,
            tc.tile_pool(name="v", bufs=2) as v_pool,
            tc.tile_pool(name="pT", bufs=40) as pT_pool,
            tc.tile_pool(name="ost", bufs=5) as o_pool,
            tc.tile_pool(name="rc", bufs=6) as rc_pool,
            tc.tile_pool(name="psbig", bufs=3, space="PSUM") as ps_big,
            tc.tile_pool(name="psctx", bufs=2, space="PSUM") as ps_ctx,
        ):
            # --- constants ---
            # per-dout-tile bias columns: bqt[:, m] = bq[128m : 128(m+1)]
            bqt = singles.tile([128, ND], F32)
            bkt = singles.tile([128, ND], F32)
            nc.gpsimd.dma_start(out=bqt, in_=bq[:].rearrange("(m p) -> p m", p=128))
            nc.gpsimd.dma_start(out=bkt, in_=bk[:].rearrange("(m p) -> p m", p=128))
            # bv broadcast to all 128 partitions (folded into V at evac)
            bvb = singles.tile([128, D], F32)
            bv_ap = bv[:]
            nc.gpsimd.dma_start(
                out=bvb,
                in_=bass.AP(tensor=bv_ap.tensor, offset=bv_ap.offset, ap=[[0, 128]] + bv_ap.ap),
            )

            # --- weights: already bf16 in DRAM, plain loads ---
            # [128, 512] half-tiles so the first Q matmuls can start after only
            # ~1 MB of wire; configs round-robin between the two HWDGE queues.
            wbf = {}
            for wname, wdram in (("q", wq), ("k", wk), ("v", wv)):
                for k in range(ND):
                    wt = wbf_pool.tile([128, D], BF16, tag=f"w{wname}{k}", name=f"w{wname}{k}")
                    wbf[(wname, k)] = wt

            def emit_w_loads():
                wq_engines = [nc.scalar, nc.sync]
                wi = 0
                for wname, wdram in (("q", wq), ("k", wk), ("v", wv)):
                    for half in range(2):
                        for k in range(ND):
                            eng = wq_engines[wi % 2]
                            wi += 1
                            eng.dma_start(
                                out=wbf[(wname, k)][:, ds(half * 512, 512)],
                                in_=wdram[ts(k, 128), ds(half * 512, 512)],
                            )

            state = {}

            def st_of(b):
                if b not in state:
                    state[b] = {"pT": {}}
                return state[b]

            # ---------- qkv units (one closure ≈ 0.9-2 µs of PE work) ----------
            def u_xt(b):
                # X^T straight from DRAM via XBAR DMA-transpose, one per din
                # block j (HWDGE-only: SP or ACT queues).
                def emit():
                    st = st_of(b)
                    st["xt"] = xt_pool.tile([128, ND * XB], BF16, tag="xt", name=f"xt{b}")
                    for j in range(ND):
                        nc.sync.dma_start_transpose(
                            out=st["xt"][:, ds(j * XB, XB)],
                            in_=hidden[b, :, ts(j, 128)],
                        )
                return emit

            def u_qk(b, which, m, part=None):
                # part=None: whole m-tile. part=0: [0:512] accumulation only
                # (allocates the psum tile). part=1: [512:S] accumulation +
                # bias evac. Splitting gives the weave finer PE granularity.
                def emit():
                    st = st_of(b)
                    key = "qt" if which == "q" else "kt"
                    if key not in st:
                        st[key] = qkt_pool.tile(
                            [128, ND * SB], BF16, tag=key, name=f"{key}{b}"
                        )
                    dst = st[key]
                    bias = bqt if which == "q" else bkt
                    xt = st["xt"]
                    if part in (None, 0):
                        ps = ps_big.tile([128, 1024], F32, tag="big", name="psbig")
                        st[("qkps", which, m)] = ps
                        for k in range(ND):
                            nc.tensor.matmul(
                                ps[:, 0:512], wbf[(which, k)][:, ts(m, 128)],
                                xt[:, ds(k * XB, 512)],
                                start=(k == 0), stop=(k == ND - 1),
                            )
                    if part in (None, 1):
                        ps = st.pop(("qkps", which, m))
                        for k in range(ND):
                            nc.tensor.matmul(
                                ps[:, 512:S], wbf[(which, k)][:, ts(m, 128)],
                                xt[:, ds(k * XB + 512, S - 512)],
                                start=(k == 0), stop=(k == ND - 1),
                            )
                        nc.vector.tensor_scalar_add(
                            dst[:, ds(m * SB, SB)], ps[:, 0:S], bias[:, m : m + 1]
                        )
                return emit

            def u_v(b, t, part=None):
                # part=0: [0:512] accumulation (allocates psum). part=1:
                # [512:1024] accumulation + bv-fold evac. None: both.
                def emit():
                    st = st_of(b)
                    if "v" not in st:
                        st["v"] = v_pool.tile([128, NT * VB], BF16, tag="v", name=f"v{b}")
                    t0, sz = S_TILES[t]
                    xt = st["xt"]
                    if part in (None, 0):
                        ps = ps_big.tile([128, 1024], F32, tag="big", name="psbig")
                        st[("vps", t)] = ps
                        for k in range(ND):
                            nc.tensor.matmul(
                                ps[:sz, 0:512], xt[:, ds(k * XB + t0, sz)],
                                wbf[("v", k)][:, 0:512],
                                start=(k == 0), stop=(k == ND - 1),
                            )
                    if part in (None, 1):
                        ps = st.pop(("vps", t))
                        for k in range(ND):
                            nc.tensor.matmul(
                                ps[:sz, 512:1024], xt[:, ds(k * XB + t0, sz)],
                                wbf[("v", k)][:, 512:1024],
                                start=(k == 0), stop=(k == ND - 1),
                            )
                        v3 = st["v"][:, ds(t * VB, VB)].rearrange("p (h c) -> p h c", c=65)
                        nc.vector.tensor_tensor(
                            out=v3[:sz, :, 0:64],
                            in0=ps[:sz].rearrange("p (h c) -> p h c", c=64),
                            in1=bvb[:sz].rearrange("p (h c) -> p h c", c=64),
                            op=OP.add,
                        )
                        nc.vector.memset(v3[:, :, 64:65], 1.0)
                return emit

            def qhead_units(b):
                # next batch's X^T load + Q projection: pure-PE filler with no
                # ACT coupling, woven into the previous batch's attn window.
                return [u_xt(b)] + [u_qk(b, "q", m) for m in range(ND)]

            # ---------- attention units ----------
            def u_sc(b, p, t, half):
                def emit():
                    st = st_of(b)
                    t0, sz = S_TILES[t]
                    h0 = half * 64
                    qt, kt = st["qt"], st["kt"]
                    ps = ps_big.tile([128, 1024], F32, tag="big", name="psbig")
                    nc.tensor.matmul(
                        ps[:sz, 0:512],
                        kt[h0 : h0 + 64, ds(p * SB + t0, sz)],
                        qt[h0 : h0 + 64, ds(p * SB, 512)],
                        start=True, stop=True, tile_position=(h0, 0),
                    )
                    nc.tensor.matmul(
                        ps[:sz, 512:S],
                        kt[h0 : h0 + 64, ds(p * SB + t0, sz)],
                        qt[h0 : h0 + 64, ds(p * SB + 512, S - 512)],
                        start=True, stop=True, tile_position=(h0, 0),
                    )
                    pT = pT_pool.tile([128, SB], BF16, tag="pT", name="pT")
                    nc.scalar.activation(pT[:sz], ps[:sz, 0:S], AF.Exp, scale=0.125)
                    st["pT"][(p, half, t)] = pT
                return emit

            def u_ctx(b, p, half=None):
                halves = (0, 1) if half is None else (half,)

                def emit():
                    st = st_of(b)
                    if "ost" not in st:
                        st["ost"] = [
                            o_pool.tile([128, D], F32, tag="ost", name=f"ost{b}_{j}")
                            for j in range(NT)
                        ]
                    for half_ in halves:
                        h = 2 * p + half_
                        psc = ps_ctx.tile([128, 512], F32, tag="ctx", name="psctx")
                        for j, (j0, sj) in enumerate(S_TILES):
                            for t, (t0, szt) in enumerate(S_TILES):
                                pT = st["pT"][(p, half_, t)]
                                nc.tensor.matmul(
                                    psc[:sj, ds(65 * j, 65)],
                                    pT[:szt, j0 : j0 + sj],
                                    st["v"][:szt, ds(t * VB + 65 * h, 65)],
                                    start=(t == 0), stop=(t == NT - 1),
                                )
                        rc = rc_pool.tile([128, 8], F32, tag="rc", name="rc")
                        den = psc[:, 0:325].rearrange("p (j c) -> p j c", c=65)
                        nc.vector.reciprocal(
                            rc[:, 0:4].rearrange("p (j c) -> p j c", c=1),
                            den[:, 0:4, 64:65],
                        )
                        nc.vector.reciprocal(
                            rc[:65, 4:5].rearrange("p (j c) -> p j c", c=1),
                            den[:65, 4:5, 64:65],
                        )
                        for j, (j0, sj) in enumerate(S_TILES):
                            nc.vector.tensor_scalar_mul(
                                st["ost"][j][:sj, ds(64 * h, 64)],
                                psc[:sj, ds(65 * j, 64)],
                                rc[:sj, j : j + 1],
                            )
                    if halves[-1] == 1:
                        # stream this head pair's 128 output columns out now
                        for j, (j0, sj) in enumerate(S_TILES):
                            nc.sync.dma_start(
                                out=out[b, j0 : j0 + sj, ds(128 * p, 128)],
                                in_=st["ost"][j][:sj, ds(128 * p, 128)],
                            )
                return emit

            def qkv_units(b, include_xt=True):
                # full projection window (batches 0..BPC-2)
                units = [u_xt(b)] if include_xt else []
                units += [u_qk(b, "q", m) for m in range(ND)]
                units += [u_qk(b, "k", m) for m in range(ND)]
                units += [u_v(b, t) for t in range(NT)]
                return units

            def attn_units(b):
                # middle-window attention: scores groups with 1-pair ctx delay
                units = []
                for p in range(HPAIRS):
                    for t in range(NT):
                        for half in range(2):
                            units.append(u_sc(b, p, t, half))
                    if p >= 1:
                        units.append(u_ctx(b, p - 1))
                units.append(u_ctx(b, HPAIRS - 1))
                return units

            # ---------- proportional weave, preserving per-list order ----------
            def weave(a_units, q_units):
                merged = []
                ia = iq = 0
                la, lq = len(a_units), len(q_units)
                while ia < la or iq < lq:
                    if iq < lq and (ia >= la or (iq + 1) * la <= (ia + 1) * lq):
                        merged.append(q_units[iq])
                        iq += 1
                    else:
                        merged.append(a_units[ia])
                        ia += 1
                return merged

            def astar_last(b):
                # final window: no next batch to weave, so this batch's own
                # K/V projections (deferred from its qkv window) act as PE
                # filler between score groups, split fine-grained.
                units = [u_qk(b, "k", 0, 0), u_qk(b, "k", 0, 1)]
                for p in range(HPAIRS):
                    scs = []
                    for t in range(NT):
                        for half in range(2):
                            scs.append(u_sc(b, p, t, half))
                    fillers = []
                    if p == 0:
                        fillers += [u_v(b, 2, 0), u_v(b, 2, 1)]
                    elif p == 1:
                        fillers += [u_v(b, 3, 0), u_v(b, 3, 1)]
                    elif p == 2:
                        fillers += [u_v(b, 4, 0), u_v(b, 4, 1)]
                    if p + 1 < HPAIRS:
                        fillers += [u_qk(b, "k", p + 1, 0), u_qk(b, "k", p + 1, 1)]
                    if p >= 3:
                        fillers += [u_ctx(b, p - 3, 0), u_ctx(b, p - 3, 1)]
                    units += weave(scs, fillers)
                for p in range(HPAIRS - 3, HPAIRS):
                    units += [u_ctx(b, p, 0), u_ctx(b, p, 1)]
                return units

            # ---------- emission ----------
            last = BPC - 1
            emit_w_loads()
            for b in range(BPC):
                if b < last:
                    attn = attn_units(b - 1) if b >= 1 else []
                    for u in weave(attn, qkv_units(b)):
                        u()
                else:
                    # next-batch head + two of its V tiles balance this window
                    qh = qhead_units(b) + [u_v(b, 0, 0), u_v(b, 0, 1),
                                           u_v(b, 1, 0), u_v(b, 1, 1)]
                    for u in weave(attn_units(b - 1), qh):
                        u()
            for u in astar_last(last):
                u()

    return nc


_NC = None


def prep_in_maps(hidden_states, Wq, bq, Wk, bk, Wv, bv):
    """Host-side prep: hidden -> bf16 zero-padded to 640 tokens; weights -> bf16."""
    import ml_dtypes

    bf16 = ml_dtypes.bfloat16
    hs = np.asarray(hidden_states, dtype=np.float32)
    hb = np.zeros((B, SP_, D), dtype=bf16)
    hb[:, :S, :] = hs.astype(bf16)
    args = {
        "Wq": np.ascontiguousarray(np.asarray(Wq, np.float32).astype(bf16)),
        "bq": np.ascontiguousarray(np.asarray(bq, np.float32)),
        "Wk": np.ascontiguousarray(np.asarray(Wk, np.float32).astype(bf16)),
        "bk": np.ascontiguousarray(np.asarray(bk, np.float32)),
        "Wv": np.ascontiguousarray(np.asarray(Wv, np.float32).astype(bf16)),
        "bv": np.ascontiguousarray(np.asarray(bv, np.float32)),
    }
    return [
        {"hidden": hb[i * BPC : (i + 1) * BPC], **args} for i in range(N_CORES)
    ]


def kernel(hidden_states, Wq, bq, Wk, bk, Wv, bv):
    global _NC
    if _NC is None:
        _NC = build_nc()
    in_maps = prep_in_maps(hidden_states, Wq, bq, Wk, bk, Wv, bv)
    res = run_bass_kernel_spmd(_NC, in_maps, list(range(N_CORES)))
    return np.concatenate([res.results[i]["out"] for i in range(N_CORES)], axis=0)


# revision 19
# speedup vs baseline: 7.6099x; 7.0661x over previous
"""ViT self-attention (B=32, S=577, D=1024, H=16, Dh=64) on 8 TRN2 NeuronCores.

Sharding: data-parallel over batch — each core gets 4 batch elements, no
collectives.

All matmuls run in bf16 (fp32 matmul is 4 cycles/row vs 1 for bf16; tolerance
2e-2 leaves ample room). The host passes hidden_states pre-converted to bf16
and zero-padded to 640 tokens, plus bf16 weights, so:
  phase 0: X^T comes straight from XBAR DMA-transpose loads (no PE transposes)
  phase 1: Q^T = Wq^T X^T, K^T = Wk^T X^T (bias folded into DVE evac, bf16),
           V natural = X Wv with bv folded in: v = [V_h + bv_h | 1] bf16 per
           head (ones column makes the softmax denominator fall out of the
           ctx matmul; (ctx + den*bv)/den = ctx/den + bv)
  phase 2: per head pair (row-packed K=64 matmuls at tile_position (0,0)/(64,0)):
           S^T tile = matmul(lhsT=K^T, rhs=Q^T); P^T = exp(S^T/8) on ACT (bf16);
           ctx natural = matmul(lhsT=P^T, rhs=[V+bv|1]) accumulated in PSUM;
           DVE: reciprocal of denominator + scale evac.
  phase 3: DMA out per head pair as soon as its ctx is evacuated.

Cross-batch software pipeline: attention of batch b (ACT-paced: exp drains
PSUM at ~154G elem/s) is woven with QKV matmuls of batch b+1 (pure PE) at
unit granularity so neither engine starves.
"""

import numpy as np

import concourse.bass as bass
import concourse.mybir as mybir
import concourse.tile as tile
from concourse.bass import ds, ts
from concourse.bass_utils import run_bass_kernel_spmd

F32 = mybir.dt.float32
BF16 = mybir.dt.bfloat16

# ---------------------------------------------------------------------------
# Wait-legalization patch: this walrus build accepts at most ONE ge-mode sync
# wait per instruction (eq-mode counts as two). Tile's sem assignment attaches
# multi-waits directly to instructions, so hoist extras onto standalone
# EventSemaphore carriers (same engine queue, immediately preceding — identical
# semantics, queue is in-order).
# ---------------------------------------------------------------------------
_ctr = [0]


def _split_waits(insts):
    out = []
    for inst in insts:
        si = inst.sync_info
        if si is not None and si.on_wait:
            waits = list(si.on_wait)
            if len(waits) == 1 and waits[0].wait_mode != "sem-eq-imm":
                move = []
            else:
                move = waits
            for w in move:
                _ctr[0] += 1
                ev = mybir.InstEventSemaphore(
                    name=f"wsplit_{_ctr[0]}", opcode="EventSemaphore",
                    engine=inst.engine, debug=inst.debug, ins=[], outs=[],
                    sync_info=mybir.SyncInfo(on_wait=[w], on_update=[]),
                )
                out.append(ev)
            if move:
                inst.sync_info = mybir.SyncInfo(on_wait=[], on_update=list(si.on_update))
        out.append(inst)
    return out


def _install_waitfix():
    if getattr(tile.TileContext, "_waitfix_installed", False):
        return
    from concourse.vector_clock import ScopedClock

    orig_lower = tile.TileContext._lower_ordered_insts

    def patched_lower(self, ordered):
        for name in list(ordered.keys()):
            ordered[name] = _split_waits(ordered[name])
        return orig_lower(self, ordered)

    def patched_dab(self, tick_clock, wait_clock):
        nc = self.nc
        probe = nc.sync.nop(nofuse=True)
        wait_clock.add_sem_waits(probe.ins, ScopedClock({None: tick_clock.global_clock}))
        si = probe.ins.sync_info
        waits = list(si.on_wait) if si is not None else []
        probe.ins.sync_info = mybir.SyncInfo(
            on_wait=[], on_update=list(si.on_update) if si else []
        )
        for w in waits:
            _ctr[0] += 1
            ev = mybir.InstEventSemaphore(
                name=f"wsplit_dab_{_ctr[0]}", opcode="EventSemaphore",
                engine=mybir.EngineType.SP, debug=probe.ins.debug, ins=[], outs=[],
                sync_info=mybir.SyncInfo(on_wait=[w], on_update=[]),
            )
            nc.sync.add_instruction(ev)
        nc.sync.drain()
        nc.all_engine_barrier()
        assert self.sems is not None
        popped = nc._tile_sem_poison_stack.pop()
        assert popped is self._sem_poison
        nc.clear_and_free_semaphores(list(self.sems.allocated().values()))
        nc.all_engine_barrier()

    tile.TileContext._lower_ordered_insts = patched_lower
    tile.TileContext._drain_and_barrier = patched_dab
    tile.TileContext._waitfix_installed = True


_install_waitfix()

N_CORES = 8
B, S, D = 32, 577, 1024
H, Dh = 16, 64
BPC = B // N_CORES  # batches per core
SP_ = 640           # padded token count (multiple of 128, for XBAR transpose)
S_TILES = [(t * 128, min(128, S - t * 128)) for t in range((S + 127) // 128)]  # 5
NT = len(S_TILES)
ND = D // 128  # 8 din/dout tiles
HPAIRS = H // 2
SB = S          # token-block stride inside fused Q^T/K^T tiles
XB = SP_        # token-block stride inside the X^T tile
VB = H * 65     # v-block stride ([V_h+bv|1] x 16 heads)

AF = mybir.ActivationFunctionType
OP = mybir.AluOpType


def build_nc(reps=1):
    nc = bass.Bass()
    hidden = nc.declare_dram_parameter("hidden", [BPC, SP_, D], BF16, isOutput=False)
    wq = nc.declare_dram_parameter("Wq", [D, D], BF16, isOutput=False)
    bq = nc.declare_dram_parameter("bq", [D], F32, isOutput=False)
    wk = nc.declare_dram_parameter("Wk", [D, D], BF16, isOutput=False)
    bk = nc.declare_dram_parameter("bk", [D], F32, isOutput=False)
    wv = nc.declare_dram_parameter("Wv", [D, D], BF16, isOutput=False)
    bv = nc.declare_dram_parameter("bv", [D], F32, isOutput=False)
    out = nc.declare_dram_parameter("out", [BPC, S, D], F32, isOutput=True)

    with tile.TileContext(nc) as tc:
        with (
            tc.tile_pool(name="singles", bufs=1) as singles,
            tc.tile_pool(name="wbf", bufs=1) as wbf_pool,
            tc.tile_pool(name="xt", bufs=2) as xt_pool,
            tc.tile_pool(name="qkt", bufs=2) as qkt_pool,
            tc.tile_pool(name="v", bufs=2) as v_pool,
            tc.tile_pool(name="pT", bufs=40) as pT_pool,
            tc.tile_pool(name="ost", bufs=5) as o_pool,
            tc.tile_pool(name="rc", bufs=6) as rc_pool,
            tc.tile_pool(name="psbig", bufs=3, space="PSUM") as ps_big,
            tc.tile_pool(name="psctx", bufs=2, space="PSUM") as ps_ctx,
        ):
            # --- constants ---
            # per-dout-tile bias columns: bqt[:, m] = bq[128m : 128(m+1)]
            bqt = singles.tile([128, ND], F32)
            bkt = singles.tile([128, ND], F32)
            nc.gpsimd.dma_start(out=bqt, in_=bq[:].rearrange("(m p) -> p m", p=128))
            nc.gpsimd.dma_start(out=bkt, in_=bk[:].rearrange("(m p) -> p m", p=128))
            # bv broadcast to all 128 partitions (folded into V at evac)
            bvb = singles.tile([128, D], F32)
            bv_ap = bv[:]
            nc.gpsimd.dma_start(
                out=bvb,
                in_=bass.AP(tensor=bv_ap.tensor, offset=bv_ap.offset, ap=[[0, 128]] + bv_ap.ap),
            )

            # --- weights: already bf16 in DRAM, plain loads ---
            # [128, 512] half-tiles so the first Q matmuls can start after only
            # ~1 MB of wire; configs round-robin between the two HWDGE queues.
            wbf = {}
            for wname, wdram in (("q", wq), ("k", wk), ("v", wv)):
                for k in range(ND):
                    wt = wbf_pool.tile([128, D], BF16, tag=f"w{wname}{k}", name=f"w{wname}{k}")
                    wbf[(wname, k)] = wt

            def emit_w_loads():
                wq_engines = [nc.scalar, nc.sync]
                wi = 0
                for wname, wdram in (("q", wq), ("k", wk), ("v", wv)):
                    for half in range(2):
                        for k in range(ND):
                            eng = wq_engines[wi % 2]
                            wi += 1
                            eng.dma_start(
                                out=wbf[(wname, k)][:, ds(half * 512, 512)],
                                in_=wdram[ts(k, 128), ds(half * 512, 512)],
                            )

            state = {}

            def st_of(b):
                if b not in state:
                    state[b] = {"pT": {}}
                return state[b]

            # ---------- qkv units (one closure ≈ 0.9-2 µs of PE work) ----------
            def u_xt(b, js=None):
                # X^T straight from DRAM via XBAR DMA-transpose, one per din
                # block j (HWDGE-only: SP or ACT queues).
                def emit():
                    st = st_of(b)
                    if "xt" not in st:
                        st["xt"] = xt_pool.tile(
                            [128, ND * XB], BF16, tag="xt", name=f"xt{b}"
                        )
                    for j in (range(ND) if js is None else js):
                        nc.sync.dma_start_transpose(
                            out=st["xt"][:, ds(j * XB, XB)],
                            in_=hidden[b, :, ts(j, 128)],
                        )
                return emit

            def u_qk(b, which, m, part=None):
                # part=None: whole m-tile. part=0: [0:512] accumulation only
                # (allocates the psum tile). part=1: [512:S] accumulation +
                # bias evac. Splitting gives the weave finer PE granularity.
                def emit():
                    st = st_of(b)
                    key = "qt" if which == "q" else "kt"
                    if key not in st:
                        st[key] = qkt_pool.tile(
                            [128, ND * SB], BF16, tag=key, name=f"{key}{b}"
                        )
                    dst = st[key]
                    bias = bqt if which == "q" else bkt
                    xt = st["xt"]
                    if part in (None, 0):
                        ps = ps_big.tile([128, 1024], F32, tag="big", name="psbig")
                        st[("qkps", which, m)] = ps
                        for k in range(ND):
                            nc.tensor.matmul(
                                ps[:, 0:512], wbf[(which, k)][:, ts(m, 128)],
                                xt[:, ds(k * XB, 512)],
                                start=(k == 0), stop=(k == ND - 1),
                            )
                    if part in (None, 1):
                        ps = st.pop(("qkps", which, m))
                        for k in range(ND):
                            nc.tensor.matmul(
                                ps[:, 512:S], wbf[(which, k)][:, ts(m, 128)],
                                xt[:, ds(k * XB + 512, S - 512)],
                                start=(k == 0), stop=(k == ND - 1),
                            )
                        nc.vector.tensor_scalar_add(
                            dst[:, ds(m * SB, SB)], ps[:, 0:S], bias[:, m : m + 1]
                        )
                return emit

            def u_v(b, t, part=None):
                # part=0: [0:512] accumulation (allocates psum). part=1:
                # [512:1024] accumulation + bv-fold evac. None: both.
                def emit():
                    st = st_of(b)
                    if "v" not in st:
                        st["v"] = v_pool.tile([128, NT * VB], BF16, tag="v", name=f"v{b}")
                    t0, sz = S_TILES[t]
                    xt = st["xt"]
                    if part in (None, 0):
                        ps = ps_big.tile([128, 1024], F32, tag="big", name="psbig")
                        st[("vps", t)] = ps
                        for k in range(ND):
                            nc.tensor.matmul(
                                ps[:sz, 0:512], xt[:, ds(k * XB + t0, sz)],
                                wbf[("v", k)][:, 0:512],
                                start=(k == 0), stop=(k == ND - 1),
                            )
                    if part in (None, 1):
                        ps = st.pop(("vps", t))
                        for k in range(ND):
                            nc.tensor.matmul(
                                ps[:sz, 512:1024], xt[:, ds(k * XB + t0, sz)],
                                wbf[("v", k)][:, 512:1024],
                                start=(k == 0), stop=(k == ND - 1),
                            )
                        v3 = st["v"][:, ds(t * VB, VB)].rearrange("p (h c) -> p h c", c=65)
                        nc.vector.tensor_tensor(
                            out=v3[:sz, :, 0:64],
                            in0=ps[:sz].rearrange("p (h c) -> p h c", c=64),
                            in1=bvb[:sz].rearrange("p (h c) -> p h c", c=64),
                            op=OP.add,
                        )
                        nc.vector.memset(v3[:, :, 64:65], 1.0)
                return emit

            def qhead_units(b):
                # next batch's X^T load + Q projection: pure-PE filler with no
                # ACT coupling, woven into the previous batch's attn window.
                return [u_xt(b)] + [u_qk(b, "q", m) for m in range(ND)]

            # ---------- attention units ----------
            def u_sc(b, p, t, half):
                def emit():
                    st = st_of(b)
                    t0, sz = S_TILES[t]
                    h0 = half * 64
                    qt, kt = st["qt"], st["kt"]
                    ps = ps_big.tile([128, 1024], F32, tag="big", name="psbig")
                    nc.tensor.matmul(
                        ps[:sz, 0:512],
                        kt[h0 : h0 + 64, ds(p * SB + t0, sz)],
                        qt[h0 : h0 + 64, ds(p * SB, 512)],
                        start=True, stop=True, tile_position=(h0, 0),
                    )
                    nc.tensor.matmul(
                        ps[:sz, 512:S],
                        kt[h0 : h0 + 64, ds(p * SB + t0, sz)],
                        qt[h0 : h0 + 64, ds(p * SB + 512, S - 512)],
                        start=True, stop=True, tile_position=(h0, 0),
                    )
                    pT = pT_pool.tile([128, SB], BF16, tag="pT", name="pT")
                    nc.scalar.activation(pT[:sz], ps[:sz, 0:S], AF.Exp, scale=0.125)
                    st["pT"][(p, half, t)] = pT
                return emit

            def u_ctx(b, p, half=None):
                halves = (0, 1) if half is None else (half,)

                def emit():
                    st = st_of(b)
                    if "ost" not in st:
                        st["ost"] = [
                            o_pool.tile([128, D], F32, tag="ost", name=f"ost{b}_{j}")
                            for j in range(NT)
                        ]
                    for half_ in halves:
                        h = 2 * p + half_
                        psc = ps_ctx.tile([128, 512], F32, tag="ctx", name="psctx")
                        for j, (j0, sj) in enumerate(S_TILES):
                            for t, (t0, szt) in enumerate(S_TILES):
                                pT = st["pT"][(p, half_, t)]
                                nc.tensor.matmul(
                                    psc[:sj, ds(65 * j, 65)],
                                    pT[:szt, j0 : j0 + sj],
                                    st["v"][:szt, ds(t * VB + 65 * h, 65)],
                                    start=(t == 0), stop=(t == NT - 1),
                                )
                        rc = rc_pool.tile([128, 8], F32, tag="rc", name="rc")
                        den = psc[:, 0:325].rearrange("p (j c) -> p j c", c=65)
                        nc.vector.reciprocal(
                            rc[:, 0:4].rearrange("p (j c) -> p j c", c=1),
                            den[:, 0:4, 64:65],
                        )
                        nc.vector.reciprocal(
                            rc[:65, 4:5].rearrange("p (j c) -> p j c", c=1),
                            den[:65, 4:5, 64:65],
                        )
                        for j, (j0, sj) in enumerate(S_TILES):
                            nc.vector.tensor_scalar_mul(
                                st["ost"][j][:sj, ds(64 * h, 64)],
                                psc[:sj, ds(65 * j, 64)],
                                rc[:sj, j : j + 1],
                            )
                    if halves[-1] == 1:
                        # stream this head pair's 128 output columns out now
                        for j, (j0, sj) in enumerate(S_TILES):
                            nc.sync.dma_start(
                                out=out[b, j0 : j0 + sj, ds(128 * p, 128)],
                                in_=st["ost"][j][:sj, ds(128 * p, 128)],
                            )
                return emit

            def qkv_units(b, include_xt=True):
                # full projection window (batches 0..BPC-2)
                units = [u_xt(b)] if include_xt else []
                units += [u_qk(b, "q", m) for m in range(ND)]
                units += [u_qk(b, "k", m) for m in range(ND)]
                units += [u_v(b, t) for t in range(NT)]
                return units

            def attn_units(b):
                # middle-window attention: scores groups with 1-pair ctx delay,
                # ctx halves interleaved among the next group's scores
                units = []
                for p in range(HPAIRS):
                    scs = []
                    for t in range(NT):
                        for half in range(2):
                            scs.append(u_sc(b, p, t, half))
                    fillers = []
                    if p >= 1:
                        fillers = [u_ctx(b, p - 1, 0), u_ctx(b, p - 1, 1)]
                    units += weave(scs, fillers)
                units += [u_ctx(b, HPAIRS - 1, 0), u_ctx(b, HPAIRS - 1, 1)]
                return units

            # ---------- proportional weave, preserving per-list order ----------
            def weave(a_units, q_units):
                merged = []
                ia = iq = 0
                la, lq = len(a_units), len(q_units)
                while ia < la or iq < lq:
                    if iq < lq and (ia >= la or (iq + 1) * la <= (ia + 1) * lq):
                        merged.append(q_units[iq])
                        iq += 1
                    else:
                        merged.append(a_units[ia])
                        ia += 1
                return merged

            def astar_last(b):
                # final window: no next batch to weave, so this batch's own
                # K/V projections (deferred from its qkv window) act as PE
                # filler between score groups, split fine-grained.
                units = [u_qk(b, "k", 0, 0), u_qk(b, "k", 0, 1)]
                for p in range(HPAIRS):
                    scs = []
                    for t in range(NT):
                        for half in range(2):
                            scs.append(u_sc(b, p, t, half))
                    fillers = []
                    if p == 0:
                        fillers += [u_v(b, 2, 0), u_v(b, 2, 1)]
                    elif p == 1:
                        fillers += [u_v(b, 3, 0), u_v(b, 3, 1)]
                    elif p == 2:
                        fillers += [u_v(b, 4, 0), u_v(b, 4, 1)]
                    if p + 1 < HPAIRS:
                        fillers += [u_qk(b, "k", p + 1, 0), u_qk(b, "k", p + 1, 1)]
                    if p >= 3:
                        fillers += [u_ctx(b, p - 3, 0), u_ctx(b, p - 3, 1)]
                    units += weave(scs, fillers)
                for p in range(HPAIRS - 3, HPAIRS):
                    units += [u_ctx(b, p, 0), u_ctx(b, p, 1)]
                return units

            # ---------- emission ----------
            # reps > 1 repeats the whole computation (weights stay resident)
            # so test.py can estimate device time differentially.
            last = BPC - 1
            emit_w_loads()
            for _rep in range(reps):
                state.clear()
                for b in range(BPC):
                    if b < last:
                        attn = attn_units(b - 1) if b >= 1 else []
                        for u in weave(attn, qkv_units(b)):
                            u()
                    else:
                        # next-batch head + two of its V tiles balance this window
                        qh = qhead_units(b) + [u_v(b, 0, 0), u_v(b, 0, 1),
                                               u_v(b, 1, 0), u_v(b, 1, 1)]
                        for u in weave(attn_units(b - 1), qh):
                            u()
                for u in astar_last(last):
                    u()

    return nc


_NC = None


def prep_in_maps(hidden_states, Wq, bq, Wk, bk, Wv, bv):
    """Host-side prep: hidden -> bf16 zero-padded to 640 tokens; weights -> bf16."""
    import ml_dtypes

    bf16 = ml_dtypes.bfloat16
    hs = np.asarray(hidden_states, dtype=np.float32)
    hb = np.zeros((B, SP_, D), dtype=bf16)
    hb[:, :S, :] = hs.astype(bf16)
    args = {
        "Wq": np.ascontiguousarray(np.asarray(Wq, np.float32).astype(bf16)),
        "bq": np.ascontiguousarray(np.asarray(bq, np.float32)),
        "Wk": np.ascontiguousarray(np.asarray(Wk, np.float32).astype(bf16)),
        "bk": np.ascontiguousarray(np.asarray(bk, np.float32)),
        "Wv": np.ascontiguousarray(np.asarray(Wv, np.float32).astype(bf16)),
        "bv": np.ascontiguousarray(np.asarray(bv, np.float32)),
    }
    return [
        {"hidden": hb[i * BPC : (i + 1) * BPC], **args} for i in range(N_CORES)
    ]


def kernel(hidden_states, Wq, bq, Wk, bk, Wv, bv):
    global _NC
    if _NC is None:
        _NC = build_nc()
    in_maps = prep_in_maps(hidden_states, Wq, bq, Wk, bk, Wv, bv)
    res = run_bass_kernel_spmd(_NC, in_maps, list(range(N_CORES)))
    return np.concatenate([res.results[i]["out"] for i in range(N_CORES)], axis=0)


# revision 28
# speedup vs baseline: 16.4865x; 2.1665x over previous
"""ViT self-attention (B=32, S=577, D=1024, H=16, Dh=64) on 8 TRN2 NeuronCores.

Sharding: data-parallel over batch — each core gets 4 batch elements, no
collectives.

All matmuls run in bf16 (fp32 matmul is 4 cycles/row vs 1 for bf16; tolerance
2e-2 leaves ample room). The host passes hidden_states pre-converted to bf16
and zero-padded to 640 tokens, plus bf16 weights, so:
  phase 0: X^T comes straight from XBAR DMA-transpose loads (no PE transposes)
  phase 1: Q^T = Wq^T X^T, K^T = Wk^T X^T (bias folded into DVE evac, bf16),
           V natural = X Wv with bv folded in: v = [V_h + bv_h | 1] bf16 per
           head (ones column makes the softmax denominator fall out of the
           ctx matmul; (ctx + den*bv)/den = ctx/den + bv)
  phase 2: per head pair (row-packed K=64 matmuls at tile_position (0,0)/(64,0)):
           S^T tile = matmul(lhsT=K^T, rhs=Q^T); P^T = exp(S^T/8) on ACT (bf16);
           ctx natural = matmul(lhsT=P^T, rhs=[V+bv|1]) accumulated in PSUM;
           DVE: reciprocal of denominator + scale evac.
  phase 3: one contiguous full-width DMA per 128-token tile (strided
           per-pair DMAs measured ~160us slower on HW: descriptor-bound).

Cross-batch software pipeline: attention of batch b (ACT-paced: exp drains
PSUM at ~154G elem/s) is woven with QKV matmuls of batch b+1 (pure PE) at
unit granularity so neither engine starves.
"""

import numpy as np

import concourse.bass as bass
import concourse.mybir as mybir
import concourse.tile as tile
from concourse.bass import ds, ts
from concourse.bass_utils import run_bass_kernel_spmd

F32 = mybir.dt.float32
BF16 = mybir.dt.bfloat16

# ---------------------------------------------------------------------------
# Wait-legalization patch: this walrus build accepts at most ONE ge-mode sync
# wait per instruction (eq-mode counts as two). Tile's sem assignment attaches
# multi-waits directly to instructions, so hoist extras onto standalone
# EventSemaphore carriers (same engine queue, immediately preceding — identical
# semantics, queue is in-order).
# ---------------------------------------------------------------------------
_ctr = [0]


def _split_waits(insts):
    out = []
    for inst in insts:
        si = inst.sync_info
        if si is not None and si.on_wait:
            waits = list(si.on_wait)
            if len(waits) == 1 and waits[0].wait_mode != "sem-eq-imm":
                move = []
            else:
                move = waits
            for w in move:
                _ctr[0] += 1
                ev = mybir.InstEventSemaphore(
                    name=f"wsplit_{_ctr[0]}", opcode="EventSemaphore",
                    engine=inst.engine, debug=inst.debug, ins=[], outs=[],
                    sync_info=mybir.SyncInfo(on_wait=[w], on_update=[]),
                )
                out.append(ev)
            if move:
                inst.sync_info = mybir.SyncInfo(on_wait=[], on_update=list(si.on_update))
        out.append(inst)
    return out


def _install_waitfix():
    if getattr(tile.TileContext, "_waitfix_installed", False):
        return
    from concourse.vector_clock import ScopedClock

    orig_lower = tile.TileContext._lower_ordered_insts

    def patched_lower(self, ordered):
        for name in list(ordered.keys()):
            ordered[name] = _split_waits(ordered[name])
        return orig_lower(self, ordered)

    def patched_dab(self, tick_clock, wait_clock):
        nc = self.nc
        probe = nc.sync.nop(nofuse=True)
        wait_clock.add_sem_waits(probe.ins, ScopedClock({None: tick_clock.global_clock}))
        si = probe.ins.sync_info
        waits = list(si.on_wait) if si is not None else []
        probe.ins.sync_info = mybir.SyncInfo(
            on_wait=[], on_update=list(si.on_update) if si else []
        )
        for w in waits:
            _ctr[0] += 1
            ev = mybir.InstEventSemaphore(
                name=f"wsplit_dab_{_ctr[0]}", opcode="EventSemaphore",
                engine=mybir.EngineType.SP, debug=probe.ins.debug, ins=[], outs=[],
                sync_info=mybir.SyncInfo(on_wait=[w], on_update=[]),
            )
            nc.sync.add_instruction(ev)
        nc.sync.drain()
        nc.all_engine_barrier()
        assert self.sems is not None
        popped = nc._tile_sem_poison_stack.pop()
        assert popped is self._sem_poison
        nc.clear_and_free_semaphores(list(self.sems.allocated().values()))
        nc.all_engine_barrier()

    tile.TileContext._lower_ordered_insts = patched_lower
    tile.TileContext._drain_and_barrier = patched_dab
    tile.TileContext._waitfix_installed = True


_install_waitfix()

N_CORES = 8
B, S, D = 32, 577, 1024
H, Dh = 16, 64
BPC = B // N_CORES  # batches per core
SP_ = 640           # padded token count (multiple of 128, for XBAR transpose)
S_TILES = [(t * 128, min(128, S - t * 128)) for t in range((S + 127) // 128)]  # 5
NT = len(S_TILES)
ND = D // 128  # 8 din/dout tiles
HPAIRS = H // 2
SB = S          # token-block stride inside fused Q^T/K^T tiles
XB = SP_        # token-block stride inside the X^T tile
VB = H * 65     # v-block stride ([V_h+bv|1] x 16 heads)

AF = mybir.ActivationFunctionType
OP = mybir.AluOpType


def build_nc(reps=1, phase="full", outdma="tile", sched="weave"):
    nc = bass.Bass()
    hidden = nc.declare_dram_parameter("hidden", [BPC, SP_, D], BF16, isOutput=False)
    wq = nc.declare_dram_parameter("Wq", [D, D], BF16, isOutput=False)
    bq = nc.declare_dram_parameter("bq", [D], F32, isOutput=False)
    wk = nc.declare_dram_parameter("Wk", [D, D], BF16, isOutput=False)
    bk = nc.declare_dram_parameter("bk", [D], F32, isOutput=False)
    wv = nc.declare_dram_parameter("Wv", [D, D], BF16, isOutput=False)
    bv = nc.declare_dram_parameter("bv", [D], F32, isOutput=False)
    out = nc.declare_dram_parameter("out", [BPC, S, D], F32, isOutput=True)

    with tile.TileContext(nc) as tc:
        with (
            tc.tile_pool(name="singles", bufs=1) as singles,
            tc.tile_pool(name="wbf", bufs=1) as wbf_pool,
            tc.tile_pool(name="xt", bufs=2) as xt_pool,
            tc.tile_pool(name="qkt", bufs=2) as qkt_pool,
            tc.tile_pool(name="v", bufs=2) as v_pool,
            tc.tile_pool(name="pT", bufs=40) as pT_pool,
            tc.tile_pool(name="ost", bufs=5) as o_pool,
            tc.tile_pool(name="rc", bufs=6) as rc_pool,
            tc.tile_pool(name="psbig", bufs=3, space="PSUM") as ps_big,
            tc.tile_pool(name="psctx", bufs=2, space="PSUM") as ps_ctx,
        ):
            # --- constants ---
            # per-dout-tile bias columns: bqt[:, m] = bq[128m : 128(m+1)]
            bqt = singles.tile([128, ND], F32)
            bkt = singles.tile([128, ND], F32)
            nc.gpsimd.dma_start(out=bqt, in_=bq[:].rearrange("(m p) -> p m", p=128))
            nc.gpsimd.dma_start(out=bkt, in_=bk[:].rearrange("(m p) -> p m", p=128))
            # bv broadcast to all 128 partitions (folded into V at evac)
            bvb = singles.tile([128, D], F32)
            bv_ap = bv[:]
            nc.gpsimd.dma_start(
                out=bvb,
                in_=bass.AP(tensor=bv_ap.tensor, offset=bv_ap.offset, ap=[[0, 128]] + bv_ap.ap),
            )

            # --- weights: already bf16 in DRAM, plain loads ---
            # [128, 512] half-tiles so the first Q matmuls can start after only
            # ~1 MB of wire; configs round-robin between the two HWDGE queues.
            wbf = {}
            for wname, wdram in (("q", wq), ("k", wk), ("v", wv)):
                for k in range(ND):
                    wt = wbf_pool.tile([128, D], BF16, tag=f"w{wname}{k}", name=f"w{wname}{k}")
                    wbf[(wname, k)] = wt

            def emit_w_loads():
                wq_engines = [nc.scalar, nc.sync]
                wi = 0
                for wname, wdram in (("q", wq), ("k", wk), ("v", wv)):
                    for half in range(2):
                        for k in range(ND):
                            eng = wq_engines[wi % 2]
                            wi += 1
                            eng.dma_start(
                                out=wbf[(wname, k)][:, ds(half * 512, 512)],
                                in_=wdram[ts(k, 128), ds(half * 512, 512)],
                            )

            state = {}

            def st_of(b):
                if b not in state:
                    state[b] = {"pT": {}}
                return state[b]

            # ---------- qkv units (one closure ≈ 0.9-2 µs of PE work) ----------
            def u_xt(b, js=None):
                # X^T straight from DRAM via XBAR DMA-transpose, one per din
                # block j (HWDGE-only: SP or ACT queues).
                def emit():
                    st = st_of(b)
                    if "xt" not in st:
                        st["xt"] = xt_pool.tile(
                            [128, ND * XB], BF16, tag="xt", name=f"xt{b}"
                        )
                    for j in (range(ND) if js is None else js):
                        nc.sync.dma_start_transpose(
                            out=st["xt"][:, ds(j * XB, XB)],
                            in_=hidden[b, :, ts(j, 128)],
                        )
                return emit

            def u_qk(b, which, m, part=None):
                # part=None: whole m-tile. part=0: [0:512] accumulation only
                # (allocates the psum tile). part=1: [512:S] accumulation +
                # bias evac. Splitting gives the weave finer PE granularity.
                def emit():
                    st = st_of(b)
                    key = "qt" if which == "q" else "kt"
                    if key not in st:
                        st[key] = qkt_pool.tile(
                            [128, ND * SB], BF16, tag=key, name=f"{key}{b}"
                        )
                    dst = st[key]
                    bias = bqt if which == "q" else bkt
                    xt = st["xt"]
                    if part in (None, 0):
                        ps = ps_big.tile([128, 1024], F32, tag="big", name="psbig")
                        st[("qkps", which, m)] = ps
                        for k in range(ND):
                            nc.tensor.matmul(
                                ps[:, 0:512], wbf[(which, k)][:, ts(m, 128)],
                                xt[:, ds(k * XB, 512)],
                                start=(k == 0), stop=(k == ND - 1),
                            )
                    if part in (None, 1):
                        ps = st.pop(("qkps", which, m))
                        for k in range(ND):
                            nc.tensor.matmul(
                                ps[:, 512:S], wbf[(which, k)][:, ts(m, 128)],
                                xt[:, ds(k * XB + 512, S - 512)],
                                start=(k == 0), stop=(k == ND - 1),
                            )
                        nc.vector.tensor_scalar_add(
                            dst[:, ds(m * SB, SB)], ps[:, 0:S], bias[:, m : m + 1]
                        )
                return emit

            def u_v(b, t, part=None):
                # part=0: [0:512] accumulation (allocates psum). part=1:
                # [512:1024] accumulation + bv-fold evac. None: both.
                def emit():
                    st = st_of(b)
                    if "v" not in st:
                        st["v"] = v_pool.tile([128, NT * VB], BF16, tag="v", name=f"v{b}")
                    t0, sz = S_TILES[t]
                    xt = st["xt"]
                    if part in (None, 0):
                        ps = ps_big.tile([128, 1024], F32, tag="big", name="psbig")
                        st[("vps", t)] = ps
                        for k in range(ND):
                            nc.tensor.matmul(
                                ps[:sz, 0:512], xt[:, ds(k * XB + t0, sz)],
                                wbf[("v", k)][:, 0:512],
                                start=(k == 0), stop=(k == ND - 1),
                            )
                    if part in (None, 1):
                        ps = st.pop(("vps", t))
                        for k in range(ND):
                            nc.tensor.matmul(
                                ps[:sz, 512:1024], xt[:, ds(k * XB + t0, sz)],
                                wbf[("v", k)][:, 512:1024],
                                start=(k == 0), stop=(k == ND - 1),
                            )
                        v3 = st["v"][:, ds(t * VB, VB)].rearrange("p (h c) -> p h c", c=65)
                        nc.vector.tensor_tensor(
                            out=v3[:sz, :, 0:64],
                            in0=ps[:sz].rearrange("p (h c) -> p h c", c=64),
                            in1=bvb[:sz].rearrange("p (h c) -> p h c", c=64),
                            op=OP.add,
                        )
                        nc.vector.memset(v3[:, :, 64:65], 1.0)
                return emit

            def qhead_units(b):
                # next batch's X^T load + Q projection: pure-PE filler with no
                # ACT coupling, woven into the previous batch's attn window.
                return [u_xt(b)] + [u_qk(b, "q", m) for m in range(ND)]

            # ---------- attention units ----------
            def u_sc(b, p, t, half):
                def emit():
                    st = st_of(b)
                    t0, sz = S_TILES[t]
                    h0 = half * 64
                    qt, kt = st["qt"], st["kt"]
                    ps = ps_big.tile([128, 1024], F32, tag="big", name="psbig")
                    nc.tensor.matmul(
                        ps[:sz, 0:512],
                        kt[h0 : h0 + 64, ds(p * SB + t0, sz)],
                        qt[h0 : h0 + 64, ds(p * SB, 512)],
                        start=True, stop=True, tile_position=(h0, 0),
                    )
                    nc.tensor.matmul(
                        ps[:sz, 512:S],
                        kt[h0 : h0 + 64, ds(p * SB + t0, sz)],
                        qt[h0 : h0 + 64, ds(p * SB + 512, S - 512)],
                        start=True, stop=True, tile_position=(h0, 0),
                    )
                    pT = pT_pool.tile([128, SB], BF16, tag="pT", name="pT")
                    nc.scalar.activation(pT[:sz], ps[:sz, 0:S], AF.Exp, scale=0.125)
                    st["pT"][(p, half, t)] = pT
                return emit

            def u_ctx(b, p, half=None):
                halves = (0, 1) if half is None else (half,)

                def emit():
                    st = st_of(b)
                    if "ost" not in st:
                        st["ost"] = [
                            o_pool.tile([128, D], F32, tag="ost", name=f"ost{b}_{j}")
                            for j in range(NT)
                        ]
                    for half_ in halves:
                        h = 2 * p + half_
                        psc = ps_ctx.tile([128, 512], F32, tag="ctx", name="psctx")
                        for j, (j0, sj) in enumerate(S_TILES):
                            for t, (t0, szt) in enumerate(S_TILES):
                                pT = st["pT"][(p, half_, t)]
                                nc.tensor.matmul(
                                    psc[:sj, ds(65 * j, 65)],
                                    pT[:szt, j0 : j0 + sj],
                                    st["v"][:szt, ds(t * VB + 65 * h, 65)],
                                    start=(t == 0), stop=(t == NT - 1),
                                )
                        rc = rc_pool.tile([128, 8], F32, tag="rc", name="rc")
                        den = psc[:, 0:325].rearrange("p (j c) -> p j c", c=65)
                        nc.vector.reciprocal(
                            rc[:, 0:4].rearrange("p (j c) -> p j c", c=1),
                            den[:, 0:4, 64:65],
                        )
                        nc.vector.reciprocal(
                            rc[:65, 4:5].rearrange("p (j c) -> p j c", c=1),
                            den[:65, 4:5, 64:65],
                        )
                        for j, (j0, sj) in enumerate(S_TILES):
                            nc.vector.tensor_scalar_mul(
                                st["ost"][j][:sj, ds(64 * h, 64)],
                                psc[:sj, ds(65 * j, 64)],
                                rc[:sj, j : j + 1],
                            )
                    if halves[-1] == 1:
                        if outdma == "pair":
                            # stream this pair's 128 output columns out now
                            for j, (j0, sj) in enumerate(S_TILES):
                                nc.sync.dma_start(
                                    out=out[b, j0 : j0 + sj, ds(128 * p, 128)],
                                    in_=st["ost"][j][:sj, ds(128 * p, 128)],
                                )
                        elif p == HPAIRS - 1:
                            # one contiguous full-width DMA per token tile
                            for j, (j0, sj) in enumerate(S_TILES):
                                nc.sync.dma_start(
                                    out=out[b, j0 : j0 + sj, :],
                                    in_=st["ost"][j][:sj],
                                )
                return emit

            def qkv_units(b, include_xt=True):
                # full projection window (batches 0..BPC-2)
                units = [u_xt(b)] if include_xt else []
                units += [u_qk(b, "q", m) for m in range(ND)]
                units += [u_qk(b, "k", m) for m in range(ND)]
                units += [u_v(b, t) for t in range(NT)]
                return units

            def attn_units(b):
                # middle-window attention: scores groups with 1-pair ctx delay,
                # ctx halves interleaved among the next group's scores
                units = []
                for p in range(HPAIRS):
                    scs = []
                    for t in range(NT):
                        for half in range(2):
                            scs.append(u_sc(b, p, t, half))
                    fillers = []
                    if p >= 1:
                        fillers = [u_ctx(b, p - 1, 0), u_ctx(b, p - 1, 1)]
                    units += weave(scs, fillers)
                units += [u_ctx(b, HPAIRS - 1, 0), u_ctx(b, HPAIRS - 1, 1)]
                return units

            # ---------- proportional weave, preserving per-list order ----------
            def weave(a_units, q_units):
                merged = []
                ia = iq = 0
                la, lq = len(a_units), len(q_units)
                while ia < la or iq < lq:
                    if iq < lq and (ia >= la or (iq + 1) * la <= (ia + 1) * lq):
                        merged.append(q_units[iq])
                        iq += 1
                    else:
                        merged.append(a_units[ia])
                        ia += 1
                return merged

            def astar_last(b):
                # final window: no next batch to weave, so this batch's own
                # K/V projections (deferred from its qkv window) act as PE
                # filler between score groups, split fine-grained.
                units = [u_qk(b, "k", 0, 0), u_qk(b, "k", 0, 1)]
                for p in range(HPAIRS):
                    scs = []
                    for t in range(NT):
                        for half in range(2):
                            scs.append(u_sc(b, p, t, half))
                    fillers = []
                    if p == 0:
                        fillers += [u_v(b, 2, 0), u_v(b, 2, 1)]
                    elif p == 1:
                        fillers += [u_v(b, 3, 0), u_v(b, 3, 1)]
                    elif p == 2:
                        fillers += [u_v(b, 4, 0), u_v(b, 4, 1)]
                    if p + 1 < HPAIRS:
                        fillers += [u_qk(b, "k", p + 1, 0), u_qk(b, "k", p + 1, 1)]
                    if p >= 3:
                        fillers += [u_ctx(b, p - 3, 0), u_ctx(b, p - 3, 1)]
                    units += weave(scs, fillers)
                for p in range(HPAIRS - 3, HPAIRS):
                    units += [u_ctx(b, p, 0), u_ctx(b, p, 1)]
                return units

            # ---------- emission ----------
            # reps > 1 repeats the whole computation (weights stay resident)
            # so test.py can estimate device time differentially.
            # phase: "full" | "qkv" (projections only, dumps V as output) |
            # "noctx" (projections + scores/exp, dumps V) — for HW bisection.
            last = BPC - 1
            emit_w_loads()
            for _rep in range(reps):
                state.clear()
                if phase == "full" and sched == "simple":
                    for b in range(BPC):
                        attn = attn_units(b - 1) if b >= 1 else []
                        for u in weave(attn, qkv_units(b)):
                            u()
                    for u in attn_units(last):
                        u()
                elif phase == "full":
                    for b in range(BPC):
                        if b < last:
                            attn = attn_units(b - 1) if b >= 1 else []
                            for u in weave(attn, qkv_units(b)):
                                u()
                        else:
                            # next-batch head + two of its V tiles balance this window
                            qh = qhead_units(b) + [u_v(b, 0, 0), u_v(b, 0, 1),
                                                   u_v(b, 1, 0), u_v(b, 1, 1)]
                            for u in weave(attn_units(b - 1), qh):
                                u()
                    for u in astar_last(last):
                        u()
                else:
                    for b in range(BPC):
                        for u in qkv_units(b):
                            u()
                        if phase == "noctx":
                            for p in range(HPAIRS):
                                for t in range(NT):
                                    for half in range(2):
                                        u_sc(b, p, t, half)()
                        # dump garbage f32 (timing only; DMA load comparable
                        # to the real out path, no cast)
                        st = st_of(b)
                        dump = o_pool.tile([128, D], F32, tag="ost", name="dump")
                        nc.vector.memset(dump, 0.0)
                        for t, (t0, sz) in enumerate(S_TILES):
                            nc.sync.dma_start(
                                out=out[b, t0 : t0 + sz, :], in_=dump[:sz]
                            )

    return nc


_NC = None


def prep_in_maps(hidden_states, Wq, bq, Wk, bk, Wv, bv):
    """Host-side prep: hidden -> bf16 zero-padded to 640 tokens; weights -> bf16."""
    import ml_dtypes

    bf16 = ml_dtypes.bfloat16
    hs = np.asarray(hidden_states, dtype=np.float32)
    hb = np.zeros((B, SP_, D), dtype=bf16)
    hb[:, :S, :] = hs.astype(bf16)
    args = {
        "Wq": np.ascontiguousarray(np.asarray(Wq, np.float32).astype(bf16)),
        "bq": np.ascontiguousarray(np.asarray(bq, np.float32)),
        "Wk": np.ascontiguousarray(np.asarray(Wk, np.float32).astype(bf16)),
        "bk": np.ascontiguousarray(np.asarray(bk, np.float32)),
        "Wv": np.ascontiguousarray(np.asarray(Wv, np.float32).astype(bf16)),
        "bv": np.ascontiguousarray(np.asarray(bv, np.float32)),
    }
    return [
        {"hidden": hb[i * BPC : (i + 1) * BPC], **args} for i in range(N_CORES)
    ]


def kernel(hidden_states, Wq, bq, Wk, bk, Wv, bv):
    global _NC
    if _NC is None:
        _NC = build_nc()
    in_maps = prep_in_maps(hidden_states, Wq, bq, Wk, bk, Wv, bv)
    res = run_bass_kernel_spmd(_NC, in_maps, list(range(N_CORES)))
    return np.concatenate([res.results[i]["out"] for i in range(N_CORES)], axis=0)


# revision 32
# speedup vs baseline: 28.1174x; 1.7055x over previous
"""ViT self-attention (B=32, S=577, D=1024, H=16, Dh=64) on 8 TRN2 NeuronCores.

Sharding: data-parallel over batch — each core gets 4 batch elements, no
collectives.

All matmuls run in bf16 (fp32 matmul is 4 cycles/row vs 1 for bf16; tolerance
2e-2 leaves ample room). The host passes hidden_states pre-TRANSPOSED to
[batch, din, token] bf16, zero-padded to 640 tokens, plus bf16 weights, so:
  phase 0: X^T tiles are plain contiguous DMAs (no PE transposes, no XBAR;
           strided/descriptor-heavy DMA patterns measured much slower on HW)
  phase 1: Q^T = Wq^T X^T, K^T = Wk^T X^T (bias folded into DVE evac, bf16),
           V natural = X Wv with bv folded in: v = [V_h + bv_h | 1] bf16 per
           head (ones column makes the softmax denominator fall out of the
           ctx matmul; (ctx + den*bv)/den = ctx/den + bv)
  phase 2: per head pair (row-packed K=64 matmuls at tile_position (0,0)/(64,0)):
           S^T tile = matmul(lhsT=K^T, rhs=Q^T); P^T = exp(S^T/8) on ACT (bf16);
           ctx natural = matmul(lhsT=P^T, rhs=[V+bv|1]) accumulated in PSUM;
           DVE: reciprocal of denominator + scale evac.
  phase 3: one contiguous full-width DMA per 128-token tile (strided
           per-pair DMAs measured ~160us slower on HW: descriptor-bound).

Cross-batch software pipeline: attention of batch b (ACT-paced: exp drains
PSUM at ~154G elem/s) is woven with QKV matmuls of batch b+1 (pure PE) at
unit granularity so neither engine starves.
"""

import numpy as np

import concourse.bass as bass
import concourse.mybir as mybir
import concourse.tile as tile
from concourse.bass import ds, ts
from concourse.bass_utils import run_bass_kernel_spmd

F32 = mybir.dt.float32
BF16 = mybir.dt.bfloat16

# ---------------------------------------------------------------------------
# Wait-legalization patch: this walrus build accepts at most ONE ge-mode sync
# wait per instruction (eq-mode counts as two). Tile's sem assignment attaches
# multi-waits directly to instructions, so hoist extras onto standalone
# EventSemaphore carriers (same engine queue, immediately preceding — identical
# semantics, queue is in-order).
# ---------------------------------------------------------------------------
_ctr = [0]


def _split_waits(insts):
    out = []
    for inst in insts:
        si = inst.sync_info
        if si is not None and si.on_wait:
            waits = list(si.on_wait)
            if len(waits) == 1 and waits[0].wait_mode != "sem-eq-imm":
                move = []
            else:
                move = waits
            for w in move:
                _ctr[0] += 1
                ev = mybir.InstEventSemaphore(
                    name=f"wsplit_{_ctr[0]}", opcode="EventSemaphore",
                    engine=inst.engine, debug=inst.debug, ins=[], outs=[],
                    sync_info=mybir.SyncInfo(on_wait=[w], on_update=[]),
                )
                out.append(ev)
            if move:
                inst.sync_info = mybir.SyncInfo(on_wait=[], on_update=list(si.on_update))
        out.append(inst)
    return out


def _install_waitfix():
    if getattr(tile.TileContext, "_waitfix_installed", False):
        return
    from concourse.vector_clock import ScopedClock

    orig_lower = tile.TileContext._lower_ordered_insts

    def patched_lower(self, ordered):
        for name in list(ordered.keys()):
            ordered[name] = _split_waits(ordered[name])
        return orig_lower(self, ordered)

    def patched_dab(self, tick_clock, wait_clock):
        nc = self.nc
        probe = nc.sync.nop(nofuse=True)
        wait_clock.add_sem_waits(probe.ins, ScopedClock({None: tick_clock.global_clock}))
        si = probe.ins.sync_info
        waits = list(si.on_wait) if si is not None else []
        probe.ins.sync_info = mybir.SyncInfo(
            on_wait=[], on_update=list(si.on_update) if si else []
        )
        for w in waits:
            _ctr[0] += 1
            ev = mybir.InstEventSemaphore(
                name=f"wsplit_dab_{_ctr[0]}", opcode="EventSemaphore",
                engine=mybir.EngineType.SP, debug=probe.ins.debug, ins=[], outs=[],
                sync_info=mybir.SyncInfo(on_wait=[w], on_update=[]),
            )
            nc.sync.add_instruction(ev)
        nc.sync.drain()
        nc.all_engine_barrier()
        assert self.sems is not None
        popped = nc._tile_sem_poison_stack.pop()
        assert popped is self._sem_poison
        nc.clear_and_free_semaphores(list(self.sems.allocated().values()))
        nc.all_engine_barrier()

    tile.TileContext._lower_ordered_insts = patched_lower
    tile.TileContext._drain_and_barrier = patched_dab
    tile.TileContext._waitfix_installed = True


_install_waitfix()

N_CORES = 8
B, S, D = 32, 577, 1024
H, Dh = 16, 64
BPC = B // N_CORES  # batches per core
SP_ = 640           # padded token count (multiple of 128, for XBAR transpose)
S_TILES = [(t * 128, min(128, S - t * 128)) for t in range((S + 127) // 128)]  # 5
NT = len(S_TILES)
ND = D // 128  # 8 din/dout tiles
HPAIRS = H // 2
SB = S          # token-block stride inside fused Q^T/K^T tiles
XB = SP_        # token-block stride inside the X^T tile
VB = H * 65     # v-block stride ([V_h+bv|1] x 16 heads)

AF = mybir.ActivationFunctionType
OP = mybir.AluOpType


def build_nc(reps=1, phase="full", outdma="tile", sched="weave"):
    nc = bass.Bass()
    # hidden arrives pre-transposed from the host: [batch, din, token] bf16,
    # so X^T tiles load as plain contiguous DMAs (the XBAR transpose path
    # reads DRAM in 256B strided chunks — descriptor-bound on HW).
    hidden = nc.declare_dram_parameter("hidden", [BPC, D, SP_], BF16, isOutput=False)
    wq = nc.declare_dram_parameter("Wq", [D, D], BF16, isOutput=False)
    bq = nc.declare_dram_parameter("bq", [D], F32, isOutput=False)
    wk = nc.declare_dram_parameter("Wk", [D, D], BF16, isOutput=False)
    bk = nc.declare_dram_parameter("bk", [D], F32, isOutput=False)
    wv = nc.declare_dram_parameter("Wv", [D, D], BF16, isOutput=False)
    bv = nc.declare_dram_parameter("bv", [D], F32, isOutput=False)
    out = nc.declare_dram_parameter("out", [BPC, S, D], F32, isOutput=True)

    with tile.TileContext(nc) as tc:
        with (
            tc.tile_pool(name="singles", bufs=1) as singles,
            tc.tile_pool(name="wbf", bufs=1) as wbf_pool,
            tc.tile_pool(name="xt", bufs=2) as xt_pool,
            tc.tile_pool(name="qkt", bufs=2) as qkt_pool,
            tc.tile_pool(name="v", bufs=2) as v_pool,
            tc.tile_pool(name="pT", bufs=40) as pT_pool,
            tc.tile_pool(name="ost", bufs=5) as o_pool,
            tc.tile_pool(name="rc", bufs=6) as rc_pool,
            tc.tile_pool(name="psbig", bufs=3, space="PSUM") as ps_big,
            tc.tile_pool(name="psctx", bufs=2, space="PSUM") as ps_ctx,
        ):
            # --- constants ---
            # per-dout-tile bias columns: bqt[:, m] = bq[128m : 128(m+1)]
            bqt = singles.tile([128, ND], F32)
            bkt = singles.tile([128, ND], F32)
            nc.gpsimd.dma_start(out=bqt, in_=bq[:].rearrange("(m p) -> p m", p=128))
            nc.gpsimd.dma_start(out=bkt, in_=bk[:].rearrange("(m p) -> p m", p=128))
            # bv broadcast to all 128 partitions (folded into V at evac)
            bvb = singles.tile([128, D], F32)
            bv_ap = bv[:]
            nc.gpsimd.dma_start(
                out=bvb,
                in_=bass.AP(tensor=bv_ap.tensor, offset=bv_ap.offset, ap=[[0, 128]] + bv_ap.ap),
            )

            # --- weights: already bf16 in DRAM, plain loads ---
            # [128, 512] half-tiles so the first Q matmuls can start after only
            # ~1 MB of wire; configs round-robin between the two HWDGE queues.
            wbf = {}
            for wname, wdram in (("q", wq), ("k", wk), ("v", wv)):
                for k in range(ND):
                    wt = wbf_pool.tile([128, D], BF16, tag=f"w{wname}{k}", name=f"w{wname}{k}")
                    wbf[(wname, k)] = wt

            def emit_w_loads():
                wq_engines = [nc.scalar, nc.sync]
                wi = 0
                for wname, wdram in (("q", wq), ("k", wk), ("v", wv)):
                    for half in range(2):
                        for k in range(ND):
                            eng = wq_engines[wi % 2]
                            wi += 1
                            eng.dma_start(
                                out=wbf[(wname, k)][:, ds(half * 512, 512)],
                                in_=wdram[ts(k, 128), ds(half * 512, 512)],
                            )

            state = {}

            def st_of(b):
                if b not in state:
                    state[b] = {"pT": {}}
                return state[b]

            # ---------- qkv units (one closure ≈ 0.9-2 µs of PE work) ----------
            def u_xt(b, js=None):
                # X^T loads: hidden is already [din, token] in DRAM, so each
                # din-block is a plain contiguous [128, 640] DMA.
                def emit():
                    st = st_of(b)
                    if "xt" not in st:
                        st["xt"] = xt_pool.tile(
                            [128, ND * XB], BF16, tag="xt", name=f"xt{b}"
                        )
                    for j in (range(ND) if js is None else js):
                        nc.sync.dma_start(
                            out=st["xt"][:, ds(j * XB, XB)],
                            in_=hidden[b, ts(j, 128), :],
                        )
                return emit

            def u_qk(b, which, m, part=None):
                # part=None: whole m-tile. part=0: [0:512] accumulation only
                # (allocates the psum tile). part=1: [512:S] accumulation +
                # bias evac. Splitting gives the weave finer PE granularity.
                def emit():
                    st = st_of(b)
                    key = "qt" if which == "q" else "kt"
                    if key not in st:
                        st[key] = qkt_pool.tile(
                            [128, ND * SB], BF16, tag=key, name=f"{key}{b}"
                        )
                    dst = st[key]
                    bias = bqt if which == "q" else bkt
                    xt = st["xt"]
                    if part in (None, 0):
                        ps = ps_big.tile([128, 1024], F32, tag="big", name="psbig")
                        st[("qkps", which, m)] = ps
                        for k in range(ND):
                            nc.tensor.matmul(
                                ps[:, 0:512], wbf[(which, k)][:, ts(m, 128)],
                                xt[:, ds(k * XB, 512)],
                                start=(k == 0), stop=(k == ND - 1),
                            )
                    if part in (None, 1):
                        ps = st.pop(("qkps", which, m))
                        for k in range(ND):
                            nc.tensor.matmul(
                                ps[:, 512:S], wbf[(which, k)][:, ts(m, 128)],
                                xt[:, ds(k * XB + 512, S - 512)],
                                start=(k == 0), stop=(k == ND - 1),
                            )
                        nc.vector.tensor_scalar_add(
                            dst[:, ds(m * SB, SB)], ps[:, 0:S], bias[:, m : m + 1]
                        )
                return emit

            def u_v(b, t, part=None):
                # part=0: [0:512] accumulation (allocates psum). part=1:
                # [512:1024] accumulation + bv-fold evac. None: both.
                def emit():
                    st = st_of(b)
                    if "v" not in st:
                        st["v"] = v_pool.tile([128, NT * VB], BF16, tag="v", name=f"v{b}")
                    t0, sz = S_TILES[t]
                    xt = st["xt"]
                    if part in (None, 0):
                        ps = ps_big.tile([128, 1024], F32, tag="big", name="psbig")
                        st[("vps", t)] = ps
                        for k in range(ND):
                            nc.tensor.matmul(
                                ps[:sz, 0:512], xt[:, ds(k * XB + t0, sz)],
                                wbf[("v", k)][:, 0:512],
                                start=(k == 0), stop=(k == ND - 1),
                            )
                    if part in (None, 1):
                        ps = st.pop(("vps", t))
                        for k in range(ND):
                            nc.tensor.matmul(
                                ps[:sz, 512:1024], xt[:, ds(k * XB + t0, sz)],
                                wbf[("v", k)][:, 512:1024],
                                start=(k == 0), stop=(k == ND - 1),
                            )
                        v3 = st["v"][:, ds(t * VB, VB)].rearrange("p (h c) -> p h c", c=65)
                        nc.vector.tensor_tensor(
                            out=v3[:sz, :, 0:64],
                            in0=ps[:sz].rearrange("p (h c) -> p h c", c=64),
                            in1=bvb[:sz].rearrange("p (h c) -> p h c", c=64),
                            op=OP.add,
                        )
                        nc.vector.memset(v3[:, :, 64:65], 1.0)
                return emit

            def qhead_units(b):
                # next batch's X^T load + Q projection: pure-PE filler with no
                # ACT coupling, woven into the previous batch's attn window.
                return [u_xt(b)] + [u_qk(b, "q", m) for m in range(ND)]

            # ---------- attention units ----------
            def u_sc(b, p, t, half):
                def emit():
                    st = st_of(b)
                    t0, sz = S_TILES[t]
                    h0 = half * 64
                    qt, kt = st["qt"], st["kt"]
                    ps = ps_big.tile([128, 1024], F32, tag="big", name="psbig")
                    nc.tensor.matmul(
                        ps[:sz, 0:512],
                        kt[h0 : h0 + 64, ds(p * SB + t0, sz)],
                        qt[h0 : h0 + 64, ds(p * SB, 512)],
                        start=True, stop=True, tile_position=(h0, 0),
                    )
                    nc.tensor.matmul(
                        ps[:sz, 512:S],
                        kt[h0 : h0 + 64, ds(p * SB + t0, sz)],
                        qt[h0 : h0 + 64, ds(p * SB + 512, S - 512)],
                        start=True, stop=True, tile_position=(h0, 0),
                    )
                    pT = pT_pool.tile([128, SB], BF16, tag="pT", name="pT")
                    nc.scalar.activation(pT[:sz], ps[:sz, 0:S], AF.Exp, scale=0.125)
                    st["pT"][(p, half, t)] = pT
                return emit

            def u_ctx(b, p, half=None):
                halves = (0, 1) if half is None else (half,)

                def emit():
                    st = st_of(b)
                    if "ost" not in st:
                        st["ost"] = [
                            o_pool.tile([128, D], F32, tag="ost", name=f"ost{b}_{j}")
                            for j in range(NT)
                        ]
                    for half_ in halves:
                        h = 2 * p + half_
                        psc = ps_ctx.tile([128, 512], F32, tag="ctx", name="psctx")
                        for j, (j0, sj) in enumerate(S_TILES):
                            for t, (t0, szt) in enumerate(S_TILES):
                                pT = st["pT"][(p, half_, t)]
                                nc.tensor.matmul(
                                    psc[:sj, ds(65 * j, 65)],
                                    pT[:szt, j0 : j0 + sj],
                                    st["v"][:szt, ds(t * VB + 65 * h, 65)],
                                    start=(t == 0), stop=(t == NT - 1),
                                )
                        rc = rc_pool.tile([128, 8], F32, tag="rc", name="rc")
                        den = psc[:, 0:325].rearrange("p (j c) -> p j c", c=65)
                        nc.vector.reciprocal(
                            rc[:, 0:4].rearrange("p (j c) -> p j c", c=1),
                            den[:, 0:4, 64:65],
                        )
                        nc.vector.reciprocal(
                            rc[:65, 4:5].rearrange("p (j c) -> p j c", c=1),
                            den[:65, 4:5, 64:65],
                        )
                        for j, (j0, sj) in enumerate(S_TILES):
                            nc.vector.tensor_scalar_mul(
                                st["ost"][j][:sj, ds(64 * h, 64)],
                                psc[:sj, ds(65 * j, 64)],
                                rc[:sj, j : j + 1],
                            )
                    if halves[-1] == 1:
                        if outdma == "pair":
                            # stream this pair's 128 output columns out now
                            for j, (j0, sj) in enumerate(S_TILES):
                                nc.sync.dma_start(
                                    out=out[b, j0 : j0 + sj, ds(128 * p, 128)],
                                    in_=st["ost"][j][:sj, ds(128 * p, 128)],
                                )
                        elif p == HPAIRS - 1:
                            # one contiguous full-width DMA per token tile
                            for j, (j0, sj) in enumerate(S_TILES):
                                nc.sync.dma_start(
                                    out=out[b, j0 : j0 + sj, :],
                                    in_=st["ost"][j][:sj],
                                )
                return emit

            def qkv_units(b, include_xt=True):
                # full projection window (batches 0..BPC-2)
                units = [u_xt(b)] if include_xt else []
                units += [u_qk(b, "q", m) for m in range(ND)]
                units += [u_qk(b, "k", m) for m in range(ND)]
                units += [u_v(b, t) for t in range(NT)]
                return units

            def attn_units(b):
                # middle-window attention: scores groups with 1-pair ctx delay,
                # ctx halves interleaved among the next group's scores
                units = []
                for p in range(HPAIRS):
                    scs = []
                    for t in range(NT):
                        for half in range(2):
                            scs.append(u_sc(b, p, t, half))
                    fillers = []
                    if p >= 1:
                        fillers = [u_ctx(b, p - 1, 0), u_ctx(b, p - 1, 1)]
                    units += weave(scs, fillers)
                units += [u_ctx(b, HPAIRS - 1, 0), u_ctx(b, HPAIRS - 1, 1)]
                return units

            # ---------- proportional weave, preserving per-list order ----------
            def weave(a_units, q_units):
                merged = []
                ia = iq = 0
                la, lq = len(a_units), len(q_units)
                while ia < la or iq < lq:
                    if iq < lq and (ia >= la or (iq + 1) * la <= (ia + 1) * lq):
                        merged.append(q_units[iq])
                        iq += 1
                    else:
                        merged.append(a_units[ia])
                        ia += 1
                return merged

            def astar_last(b):
                # final window: no next batch to weave, so this batch's own
                # K/V projections (deferred from its qkv window) act as PE
                # filler between score groups, split fine-grained.
                units = [u_qk(b, "k", 0, 0), u_qk(b, "k", 0, 1)]
                for p in range(HPAIRS):
                    scs = []
                    for t in range(NT):
                        for half in range(2):
                            scs.append(u_sc(b, p, t, half))
                    fillers = []
                    if p == 0:
                        fillers += [u_v(b, 2, 0), u_v(b, 2, 1)]
                    elif p == 1:
                        fillers += [u_v(b, 3, 0), u_v(b, 3, 1)]
                    elif p == 2:
                        fillers += [u_v(b, 4, 0), u_v(b, 4, 1)]
                    if p + 1 < HPAIRS:
                        fillers += [u_qk(b, "k", p + 1, 0), u_qk(b, "k", p + 1, 1)]
                    if p >= 3:
                        fillers += [u_ctx(b, p - 3, 0), u_ctx(b, p - 3, 1)]
                    units += weave(scs, fillers)
                for p in range(HPAIRS - 3, HPAIRS):
                    units += [u_ctx(b, p, 0), u_ctx(b, p, 1)]
                return units

            # ---------- emission ----------
            # reps > 1 repeats the whole computation (weights stay resident)
            # so test.py can estimate device time differentially.
            # phase: "full" | "qkv" (projections only, dumps V as output) |
            # "noctx" (projections + scores/exp, dumps V) — for HW bisection.
            last = BPC - 1
            emit_w_loads()
            for _rep in range(reps):
                state.clear()
                if phase == "full" and sched == "simple":
                    for b in range(BPC):
                        attn = attn_units(b - 1) if b >= 1 else []
                        for u in weave(attn, qkv_units(b)):
                            u()
                    for u in attn_units(last):
                        u()
                elif phase == "full":
                    for b in range(BPC):
                        if b < last:
                            attn = attn_units(b - 1) if b >= 1 else []
                            for u in weave(attn, qkv_units(b)):
                                u()
                        else:
                            # next-batch head + two of its V tiles balance this window
                            qh = qhead_units(b) + [u_v(b, 0, 0), u_v(b, 0, 1),
                                                   u_v(b, 1, 0), u_v(b, 1, 1)]
                            for u in weave(attn_units(b - 1), qh):
                                u()
                    for u in astar_last(last):
                        u()
                else:
                    for b in range(BPC):
                        for u in qkv_units(b):
                            u()
                        if phase == "noctx":
                            for p in range(HPAIRS):
                                for t in range(NT):
                                    for half in range(2):
                                        u_sc(b, p, t, half)()
                        # dump garbage f32 (timing only; DMA load comparable
                        # to the real out path, no cast)
                        st = st_of(b)
                        dump = o_pool.tile([128, D], F32, tag="ost", name="dump")
                        nc.vector.memset(dump, 0.0)
                        for t, (t0, sz) in enumerate(S_TILES):
                            nc.sync.dma_start(
                                out=out[b, t0 : t0 + sz, :], in_=dump[:sz]
                            )

    return nc


_NC = None


def prep_in_maps(hidden_states, Wq, bq, Wk, bk, Wv, bv):
    """Host-side prep: hidden -> bf16 zero-padded to 640 tokens; weights -> bf16."""
    import ml_dtypes

    bf16 = ml_dtypes.bfloat16
    hs = np.asarray(hidden_states, dtype=np.float32)
    hb = np.zeros((B, D, SP_), dtype=bf16)
    hb[:, :, :S] = hs.transpose(0, 2, 1).astype(bf16)
    args = {
        "Wq": np.ascontiguousarray(np.asarray(Wq, np.float32).astype(bf16)),
        "bq": np.ascontiguousarray(np.asarray(bq, np.float32)),
        "Wk": np.ascontiguousarray(np.asarray(Wk, np.float32).astype(bf16)),
        "bk": np.ascontiguousarray(np.asarray(bk, np.float32)),
        "Wv": np.ascontiguousarray(np.asarray(Wv, np.float32).astype(bf16)),
        "bv": np.ascontiguousarray(np.asarray(bv, np.float32)),
    }
    return [
        {"hidden": hb[i * BPC : (i + 1) * BPC], **args} for i in range(N_CORES)
    ]


def kernel(hidden_states, Wq, bq, Wk, bk, Wv, bv):
    global _NC
    if _NC is None:
        _NC = build_nc()
    in_maps = prep_in_maps(hidden_states, Wq, bq, Wk, bk, Wv, bv)
    res = run_bass_kernel_spmd(_NC, in_maps, list(range(N_CORES)))
    return np.concatenate([res.results[i]["out"] for i in range(N_CORES)], axis=0)
